# revision 1
# baseline (speedup 1.0000x reference)
"""BasicTransformerBlock (self-attn + cross-attn + GEGLU FF) on 8 TRN2 cores.

Sharding: sequence-parallel, no collectives. B=4 batches x 2 sequence-halves
= 8 shards; each core computes 512 query rows end-to-end, duplicating only
the (cheap) K/V projections for its batch. The host rolls each batch's
hidden_states so a core's query rows are always rows 0..511 — the kernel is
uniform SPMD.

Device dataflow per core (token-major "natural" layout [s-part, d-free]):
  - LN stats via DVE bn_stats/bn_aggr; rstd = 1/sqrt(var+eps) via ACT Sqrt +
    DVE reciprocal (keeps ACT table switches to one per phase)
  - normalized activations PE-transposed to [d-part, s-free] once per block
  - Q^T/K^T projected in transposed form (lhsT=W, rhs=xn^T); V natural
  - scores^T per head in PSUM -> ACT exp -> E^T (bf16)
  - PV with fused softmax denominator: rhs = [V_h | ones] so one matmul gives
    both E@V and the row sums; per-partition reciprocal normalizes in natural
    layout.  LN gamma/beta folded into the following weights host-side; the
    key-padding mask folded into V2 rows + ones column (exp(-inf) == 0).
  - all big matmuls in float32r (full-rate fp32, ~1e-4 rel err); PV in bf16
"""

import contextlib
import os

import numpy as np

_KSTOP = int(os.environ.get("KSTOP", "99"))

import concourse.mybir as mybir
import concourse.tile as tile
from concourse import bacc
from concourse.bass_utils import run_bass_kernel_spmd
from concourse.masks import make_identity

P = 128
B, S, T, D, H, DH = 4, 1024, 1024, 1024, 16, 64
FF = 4 * D
SQ = 512                 # query rows per core
SCALE = DH ** -0.5
EPS = 1e-12
NCORES = 8

f32 = mybir.dt.float32
f32r = mybir.dt.float32r
bf16 = mybir.dt.bfloat16
AF = mybir.ActivationFunctionType
ALU = mybir.AluOpType
AX = mybir.AxisListType

DSUB = D // P            # 8
TSUB = T // P            # 8
SSUB = S // P            # 8
QSUB = SQ // P           # 4
FSUB = FF // P           # 32


# --------------------------------------------------------------------------
# device-program helpers
# --------------------------------------------------------------------------

def _ln_tile(nc, sb_small, x_ap, xn_ap, eps_ap):
    """xn = (x - mean) * rsqrt(var + eps) via DVE bn_stats/bn_aggr."""
    stats = sb_small.tile([P, D // 512, 6], f32, tag="ln_stats")
    for c in range(D // 512):
        nc.vector.bn_stats(stats[:, c], x_ap[:, c * 512:(c + 1) * 512])
    mv = sb_small.tile([P, 2], f32, tag="ln_mv")
    nc.vector.bn_aggr(mv, stats)
    std = sb_small.tile([P, 1], f32, tag="ln_std")
    nc.scalar.activation(std, mv[:, 1:2], AF.Sqrt, bias=eps_ap)
    rstd = sb_small.tile([P, 1], f32, tag="ln_rstd")
    nc.vector.reciprocal(rstd, std)
    for c in range(2):
        sl = slice(c * (D // 2), (c + 1) * (D // 2))
        nc.vector.tensor_scalar(xn_ap[:, sl], x_ap[:, sl], mv[:, 0:1], rstd,
                                ALU.subtract, ALU.mult)


def _transpose_in(nc, ps_tr, ident, src, dst, n_scols):
    """PE-transpose src [128, n_scols*128 (s-part, d-free)] into dst slices.

    src: natural tile AP [P, n_dcols*P]; dst: xT tile [P, DSUB, ...] with
    column range cols (s block).
    """
    tp = ps_tr.tile([P, P], f32, tag="tr_ps")
    nc.tensor.transpose(tp, src, ident)
    nc.vector.tensor_copy(dst, tp)


NWQ = 4  # weight quarters


def _load_w_halves(nc, wpool, w_dr):
    """Stream a [D, D] weight as NWQ [128, 8//NWQ, D] tiles (k = ks*128+p)."""
    step = DSUB // NWQ
    parts = []
    for q in range(NWQ):
        wt = wpool.tile([P, step, D], f32r, tag="w")
        nc.sync.dma_start(
            wt,
            w_dr.rearrange("(ks p) o -> p ks o", p=P)[:, q * step:(q + 1) * step])
        parts.append(wt)
    return parts


def _proj_T(nc, wpool, ps_pool, w_dr, rhsT, outT, bias_s, ncols):
    """outT[:, dsb, :] = (W.T @ xn)[d-chunk dsb] over `ncols` s/t columns.

    rhsT: [P, DSUB, >=ncols] f32r; outT: [P, DSUB, ncols] f32r;
    bias_s: [P, DSUB] per-output-channel bias or None.
    """
    halves = _load_w_halves(nc, wpool, w_dr)
    nhalf = ncols // 512
    for dsb in range(DSUB):
        for ch in range(nhalf):
            ps = ps_pool.tile([P, 512], f32, tag="proj")
            for ks in range(DSUB):
                nc.tensor.matmul(
                    ps,
                    halves[ks // (DSUB // NWQ)][:, ks % (DSUB // NWQ),
                                                dsb * P:(dsb + 1) * P],
                    rhsT[:, ks, ch * 512:(ch + 1) * 512],
                    start=(ks == 0), stop=(ks == DSUB - 1),
                )
            dst = outT[:, dsb, ch * 512:(ch + 1) * 512]
            if bias_s is not None:
                nc.scalar.activation(
                    dst, ps, AF.Identity, bias=bias_s[:, dsb:dsb + 1])
            else:
                nc.scalar.copy(dst, ps)


def _proj_v(nc, wpool, ps_pool, w_dr, lhsT, v_out, vb_b, mask_s):
    """V[t, dv] natural, written per head with a fused 65th column.

    v_out: [P, TSUB, H, 65] bf16.  col 64 = ones (vb_b path) or mask value.
    If mask_s is given, V rows are scaled by the mask (exp(s+log m) == e*m).
    """
    halves = _load_w_halves(nc, wpool, w_dr)
    for ts in range(TSUB):
        for dh in range(2):
            ps = ps_pool.tile([P, 512], f32, tag="proj")
            for ks in range(DSUB):
                nc.tensor.matmul(
                    ps,
                    lhsT[:, ks, ts * P:(ts + 1) * P],
                    halves[ks // (DSUB // NWQ)][:, ks % (DSUB // NWQ),
                                                dh * 512:(dh + 1) * 512],
                    start=(ks == 0), stop=(ks == DSUB - 1),
                )
            dst = v_out[:, ts, dh * 8:(dh + 1) * 8, 0:64]
            src = ps.rearrange("p (h w) -> p h w", h=8)
            if mask_s is None:
                nc.vector.tensor_tensor(
                    dst, src,
                    vb_b[:, dh * 512:(dh + 1) * 512].rearrange(
                        "p (h w) -> p h w", h=8),
                    ALU.add)
            else:
                nc.scalar.mul(dst, src, mask_s[:, ts:ts + 1])
    if mask_s is None:
        nc.vector.memset(v_out[:, :, :, 64:65], 1.0)
    else:
        for ts in range(TSUB):
            nc.vector.tensor_copy(
                v_out[:, ts, :, 64],
                mask_s[:, ts:ts + 1].to_broadcast((P, H)))


def _attention(nc, ET_pool, ps_sc, ps_pv, sb_small, qT, kT, v, attn_out):
    """Multi-head attention: scores^T -> exp -> PV with fused denominator."""
    for h in range(H):
        p0 = (h % 2) * 64
        ds = h // 2
        ET = ET_pool.tile([P, TSUB, SQ], bf16, tag="ET")
        for grp in range(TSUB // 2):
            ps = ps_sc.tile([P, 2, SQ], f32, tag="sc")
            for c2 in range(2):
                t_i = grp * 2 + c2
                nc.tensor.matmul(
                    ps[:, c2],
                    kT[p0:p0 + 64, ds, t_i * P:(t_i + 1) * P],
                    qT[p0:p0 + 64, ds, :],
                    start=True, stop=True,
                )
            nc.scalar.activation(
                ET[:, grp * 2:(grp + 1) * 2, :], ps, AF.Exp, scale=SCALE)
        for sc in range(QSUB):
            pv = ps_pv.tile([P, 65], f32, tag="pv")
            for ts in range(TSUB):
                nc.tensor.matmul(
                    pv,
                    ET[:, ts, sc * P:(sc + 1) * P],
                    v[:, ts, h],
                    start=(ts == 0), stop=(ts == TSUB - 1),
                )
            rec = sb_small.tile([P, 1], f32, tag="pv_rec")
            nc.vector.reciprocal(rec, pv[:, 64:65])
            nc.vector.tensor_scalar(
                attn_out[:, sc, h * DH:(h + 1) * DH], pv[:, 0:64], rec, None,
                ALU.mult)


def _out_proj(nc, tc, sb_small, w_dr, aoutT, bias_b, resid, h_out):
    """h_out = resid + aout @ Wo + bias."""
    with (
        tc.tile_pool(name="wo_w", bufs=5) as wpool,
        tc.tile_pool(name="wo_ps", bufs=3, space="PSUM") as ps_pool,
        tc.tile_pool(name="wo_tmp", bufs=3) as tmp_pool,
    ):
        halves = _load_w_halves(nc, wpool, w_dr)
        for sc in range(QSUB):
            for dh in range(2):
                ps = ps_pool.tile([P, 512], f32, tag="wo")
                for ks in range(DSUB):
                    nc.tensor.matmul(
                        ps,
                        aoutT[:, ks, sc * P:(sc + 1) * P],
                        halves[ks // (DSUB // NWQ)][:, ks % (DSUB // NWQ),
                                                    dh * 512:(dh + 1) * 512],
                        start=(ks == 0), stop=(ks == DSUB - 1),
                    )
                t1 = tmp_pool.tile([P, 512], f32, tag="wo_t1")
                nc.vector.tensor_add(t1, ps, resid[:, sc, dh * 512:(dh + 1) * 512])
                nc.vector.tensor_add(
                    h_out[:, sc, dh * 512:(dh + 1) * 512], t1,
                    bias_b[:, dh * 512:(dh + 1) * 512])


def _ln_transpose_q(nc, tc, sb_small, ident, h_in, xnT, eps_ap):
    """LN each of the 4 h-chunks and transpose into xnT [P, DSUB, SQ]."""
    with (
        tc.tile_pool(name="lnq", bufs=4) as xn_pool,
        tc.tile_pool(name="lnq_tr", bufs=6, space="PSUM") as ps_tr,
    ):
        for sc in range(QSUB):
            xn = xn_pool.tile([P, D], f32, tag="xn")
            _ln_tile(nc, sb_small, h_in[:, sc], xn, eps_ap)
            for dsb in range(DSUB):
                _transpose_in(
                    nc, ps_tr, ident, xn[:, dsb * P:(dsb + 1) * P],
                    xnT[:, dsb, sc * P:(sc + 1) * P], dsb)


def _transpose_aout(nc, tc, ident, attn_out, aoutT):
    with tc.tile_pool(name="aout_tr", bufs=8, space="PSUM") as ps_tr:
        for sc in range(QSUB):
            for dsb in range(DSUB):
                _transpose_in(
                    nc, ps_tr, ident, attn_out[:, sc, dsb * P:(dsb + 1) * P],
                    aoutT[:, dsb, sc * P:(sc + 1) * P], dsb)


# --------------------------------------------------------------------------
# full program
# --------------------------------------------------------------------------

def build_nc(reps=1):
    nc = bacc.Bacc(None, target_bir_lowering=False, debug=False)

    x_dr = nc.dram_tensor("x", [S, D], f32, kind="ExternalInput")
    ctxT_dr = nc.dram_tensor("ctxT", [D, T], f32r, kind="ExternalInput")
    mask_dr = nc.dram_tensor("mask_f", [T], f32, kind="ExternalInput")
    wdr = {}
    for a in (1, 2):
        for nm in ("Wq", "Wk", "Wv", "Wo"):
            wdr[f"{nm}{a}"] = nc.dram_tensor(
                f"{nm}{a}", [D, D], f32r, kind="ExternalInput")
    qb1_dr = nc.dram_tensor("qb1", [D], f32, kind="ExternalInput")
    kb1_dr = nc.dram_tensor("kb1", [D], f32, kind="ExternalInput")
    vb1_dr = nc.dram_tensor("vb1", [D], f32, kind="ExternalInput")
    bo1_dr = nc.dram_tensor("bo1", [D], f32, kind="ExternalInput")
    qb2_dr = nc.dram_tensor("qb2", [D], f32, kind="ExternalInput")
    bo2_dr = nc.dram_tensor("bo2", [D], f32, kind="ExternalInput")
    wff1_dr = nc.dram_tensor("Wff1", [D, 2 * FF], f32r, kind="ExternalInput")
    bff1_dr = nc.dram_tensor("bff1", [2 * FF], f32, kind="ExternalInput")
    wff2_dr = nc.dram_tensor("Wff2", [FF, D], f32r, kind="ExternalInput")
    bff2_dr = nc.dram_tensor("bff2", [D], f32, kind="ExternalInput")
    out_dr = nc.dram_tensor("out", [SQ, D], f32, kind="ExternalOutput")

    x_tiled = x_dr.rearrange("(ss p) d -> p ss d", p=P)

    with tile.TileContext(nc) as tc, contextlib.ExitStack() as es:
        const = es.enter_context(tc.tile_pool(name="const", bufs=1))
        sb_small = es.enter_context(tc.tile_pool(name="smalls", bufs=6))

        ident = const.tile([P, P], f32)
        make_identity(nc, ident)
        eps_ap = const.tile([P, 1], f32)
        nc.vector.memset(eps_ap, EPS)
        bo1_b = const.tile([P, D], f32)
        nc.sync.dma_start(bo1_b, bo1_dr[None, :].to_broadcast((P, D)))
        bo2_b = const.tile([P, D], f32)
        nc.sync.dma_start(bo2_b, bo2_dr[None, :].to_broadcast((P, D)))
        bff2_b = const.tile([P, D], f32)
        nc.sync.dma_start(bff2_b, bff2_dr[None, :].to_broadcast((P, D)))
        vb1_b = const.tile([P, D], f32)
        nc.sync.dma_start(vb1_b, vb1_dr[None, :].to_broadcast((P, D)))
        qb1_s = const.tile([P, DSUB], f32)
        nc.sync.dma_start(qb1_s, qb1_dr.rearrange("(c p) -> p c", p=P))
        kb1_s = const.tile([P, DSUB], f32)
        nc.sync.dma_start(kb1_s, kb1_dr.rearrange("(c p) -> p c", p=P))
        qb2_s = const.tile([P, DSUB], f32)
        nc.sync.dma_start(qb2_s, qb2_dr.rearrange("(c p) -> p c", p=P))
        bff1_s = const.tile([P, 2 * FSUB], f32)
        nc.sync.dma_start(bff1_s, bff1_dr.rearrange("(c p) -> p c", p=P))
        mask_s = const.tile([P, TSUB], f32)
        nc.sync.dma_start(mask_s, mask_dr.rearrange("(c p) -> p c", p=P))

        for _rep in range(reps):
            # Residual buffer, reused in place: x_q -> h1 -> h2 -> out.
            hbuf, free_hbuf = tc.tile([P, QSUB, D], f32, name="hbuf")
            for sc in range(QSUB):
                nc.sync.dma_start(hbuf[:, sc], x_tiled[:, sc])

            # ---- Phase 1: LN1 + transpose ----
            xn1T, free_xn1T = tc.tile([P, DSUB, S], f32r, name="xn1T")
            w1_es = contextlib.ExitStack()
            wpool1 = w1_es.enter_context(tc.tile_pool(name="w1", bufs=5))
            if _KSTOP >= 1:
             with (
                nc.named_scope("ln1"),
                tc.tile_pool(name="x_hi", bufs=3) as x_hi_pool,
                tc.tile_pool(name="xn1", bufs=4) as xn1_pool,
                tc.tile_pool(name="tr1_ps", bufs=6, space="PSUM") as ps_tr,
            ):
                for ss in range(SSUB):
                    if ss < QSUB:
                        xt = hbuf[:, ss]
                    else:
                        xt = x_hi_pool.tile([P, D], f32, tag="x_hi")
                        nc.sync.dma_start(xt, x_tiled[:, ss])
                    xn = xn1_pool.tile([P, D], f32, tag="xn1")
                    _ln_tile(nc, sb_small, xt, xn, eps_ap)
                    for dsb in range(DSUB):
                        _transpose_in(
                            nc, ps_tr, ident, xn[:, dsb * P:(dsb + 1) * P],
                            xn1T[:, dsb, ss * P:(ss + 1) * P], 1)

            # ---- attention block 1 (self): QKV -> attn -> out-proj (into hbuf) --
            attn1, free_attn1 = tc.tile([P, QSUB, D], f32, name="attn1")
            q1T, free_q1T = tc.tile([P, DSUB, SQ], f32r, name="q1T")
            k1T, free_k1T = tc.tile([P, DSUB, T], f32r, name="k1T")
            v1, free_v1 = tc.tile([P, TSUB, H, 65], bf16, name="v1")
            if _KSTOP >= 2:
             with (
                nc.named_scope("qkv1"),
                tc.tile_pool(name="qkv1_ps", bufs=6, space="PSUM") as ps_proj,
            ):
                _proj_T(nc, wpool1, ps_proj, wdr["Wq1"], xn1T, q1T, qb1_s, SQ)
                _proj_T(nc, wpool1, ps_proj, wdr["Wk1"], xn1T, k1T, kb1_s, T)
                _proj_v(nc, wpool1, ps_proj, wdr["Wv1"], xn1T, v1, vb1_b, None)
            if _KSTOP >= 3:
             with (
                nc.named_scope("attn1"),
                tc.tile_pool(name="ET1", bufs=2) as ET_pool,
                tc.tile_pool(name="sc1_ps", bufs=2, space="PSUM") as ps_sc,
                tc.tile_pool(name="pv1_ps", bufs=4, space="PSUM") as ps_pv,
            ):
                _attention(nc, ET_pool, ps_sc, ps_pv, sb_small, q1T, k1T, v1, attn1)
            free_v1(); free_k1T(); free_q1T()
            aout1T, free_aout1T = tc.tile([P, DSUB, SQ], f32r, name="aout1T")
            if _KSTOP >= 4:
             with nc.named_scope("wo1"):
                _transpose_aout(nc, tc, ident, attn1, aout1T)
                _out_proj(nc, tc, sb_small, wdr["Wo1"], aout1T, bo1_b, hbuf, hbuf)
            free_aout1T(); free_attn1(); w1_es.close(); free_xn1T()

            # ---- attention block 2 (cross) ----
            xn2T, free_xn2T = tc.tile([P, DSUB, SQ], f32r, name="xn2T")
            w2_es = contextlib.ExitStack()
            wpool2 = w2_es.enter_context(tc.tile_pool(name="w2", bufs=5))
            if _KSTOP >= 5:
             with nc.named_scope("ln2"):
                _ln_transpose_q(nc, tc, sb_small, ident, hbuf, xn2T, eps_ap)
            attn2, free_attn2 = tc.tile([P, QSUB, D], f32, name="attn2")
            q2T, free_q2T = tc.tile([P, DSUB, SQ], f32r, name="q2T")
            k2T, free_k2T = tc.tile([P, DSUB, T], f32r, name="k2T")
            v2, free_v2 = tc.tile([P, TSUB, H, 65], bf16, name="v2")
            ctxT_sb, free_ctxT = tc.tile([P, DSUB, T], f32r, name="ctxT_sb")
            _ctxT_t = ctxT_dr.rearrange("(ds p) t -> p ds t", p=P)
            for ds in range(DSUB):
                nc.sync.dma_start(ctxT_sb[:, ds], _ctxT_t[:, ds])
            if _KSTOP >= 6:
             with (
                nc.named_scope("qkv2"),
                tc.tile_pool(name="qkv2_ps", bufs=6, space="PSUM") as ps_proj,
            ):
                _proj_T(nc, wpool2, ps_proj, wdr["Wq2"], xn2T, q2T, qb2_s, SQ)
                _proj_T(nc, wpool2, ps_proj, wdr["Wk2"], ctxT_sb, k2T, None, T)
                _proj_v(nc, wpool2, ps_proj, wdr["Wv2"], ctxT_sb, v2, None, mask_s)
            free_ctxT()
            if _KSTOP >= 7:
             with (
                nc.named_scope("attn2"),
                tc.tile_pool(name="ET2", bufs=2) as ET_pool,
                tc.tile_pool(name="sc2_ps", bufs=2, space="PSUM") as ps_sc,
                tc.tile_pool(name="pv2_ps", bufs=4, space="PSUM") as ps_pv,
            ):
                _attention(nc, ET_pool, ps_sc, ps_pv, sb_small, q2T, k2T, v2, attn2)
            free_v2(); free_k2T(); free_q2T()
            aout2T, free_aout2T = tc.tile([P, DSUB, SQ], f32r, name="aout2T")
            if _KSTOP >= 8:
             with nc.named_scope("wo2"):
                _transpose_aout(nc, tc, ident, attn2, aout2T)
                _out_proj(nc, tc, sb_small, wdr["Wo2"], aout2T, bo2_b, hbuf, hbuf)
            free_aout2T(); free_attn2(); w2_es.close(); free_xn2T()

            # ---- GEGLU feed-forward ----
            xn3T, free_xn3T = tc.tile([P, DSUB, SQ], f32r, name="xn3T")
            if _KSTOP >= 9:
             with nc.named_scope("ln3"):
                _ln_transpose_q(nc, tc, sb_small, ident, hbuf, xn3T, eps_ap)

            mT, free_mT = tc.tile([P, FSUB, SQ], f32r, name="mT")
            ff_es = contextlib.ExitStack()
            wff2_pool = ff_es.enter_context(tc.tile_pool(name="wff2", bufs=10))
            wff1_t = wff1_dr.rearrange("(ks p) f -> p ks f", p=P)
            if _KSTOP >= 10:
             with (
                nc.named_scope("ff1"),
                tc.tile_pool(name="wff1", bufs=8) as wff1_pool,
                tc.tile_pool(name="ff1_ps", bufs=6, space="PSUM") as ps_ff1,
                tc.tile_pool(name="hT", bufs=3) as hT_pool,
                tc.tile_pool(name="gT", bufs=3) as gT_pool,
            ):
                for fc in range(FSUB):
                    hg = []
                    for part_i, fg in ((0, fc), (1, fc + FSUB)):
                        wt = wff1_pool.tile([P, DSUB, P], f32r, tag="wff1")
                        nc.sync.dma_start(wt, wff1_t[:, :, fg * P:(fg + 1) * P])
                        ps = ps_ff1.tile([P, SQ], f32, tag="yT")
                        for ks in range(DSUB):
                            nc.tensor.matmul(
                                ps, wt[:, ks], xn3T[:, ks, :],
                                start=(ks == 0), stop=(ks == DSUB - 1))
                        if part_i == 0:
                            hT = hT_pool.tile([P, SQ], f32, tag="hT")
                            nc.vector.tensor_scalar(
                                hT, ps, bff1_s[:, fg:fg + 1], None, ALU.add)
                            hg.append(hT)
                        else:
                            gT = gT_pool.tile([P, SQ], f32, tag="gT")
                            nc.scalar.activation(
                                gT, ps, AF.Gelu, bias=bff1_s[:, fg:fg + 1])
                            hg.append(gT)
                    nc.vector.tensor_tensor(mT[:, fc, :], hg[0], hg[1], ALU.mult)

            wff2_t = wff2_dr.rearrange("(ks p) o -> p ks o", p=P)
            if _KSTOP >= 11:
             with (
                nc.named_scope("ff2"),
                tc.tile_pool(name="ff2_ps", bufs=1, space="PSUM") as ps_ff2,
                tc.tile_pool(name="ff2_tmp", bufs=3) as tmp_pool,
            ):
                ps_o = [ps_ff2.tile([P, 512], f32, tag=f"o{i}", name=f"ps_o{i}")
                        for i in range(8)]
                for ks in range(FSUB):
                    wt = wff2_pool.tile([P, D], f32r, tag="wff2")
                    nc.sync.dma_start(wt, wff2_t[:, ks])
                    for sc in range(QSUB):
                        for dh in range(2):
                            nc.tensor.matmul(
                                ps_o[sc * 2 + dh],
                                mT[:, ks, sc * P:(sc + 1) * P],
                                wt[:, dh * 512:(dh + 1) * 512],
                                start=(ks == 0), stop=(ks == FSUB - 1))
                for sc in range(QSUB):
                    for dh in range(2):
                        sl = slice(dh * 512, (dh + 1) * 512)
                        t1 = tmp_pool.tile([P, 512], f32, tag="ff2_t1")
                        nc.vector.tensor_add(t1, ps_o[sc * 2 + dh], hbuf[:, sc, sl])
                        nc.vector.tensor_add(
                            hbuf[:, sc, sl], t1, bff2_b[:, sl])
            ff_es.close(); free_mT(); free_xn3T()

            nc.sync.dma_start(out_dr.rearrange("(ss p) d -> p ss d", p=P), hbuf)
            free_hbuf()

    nc.compile()
    return nc


# --------------------------------------------------------------------------
# host side
# --------------------------------------------------------------------------

_NC = None


def _get_nc():
    global _NC
    if _NC is None:
        _NC = build_nc()
    return _NC


def _make_in_maps(inputs):
    f = np.float32
    hidden = np.asarray(inputs["hidden_states"], f)
    context = np.asarray(inputs["context"], f)
    mask = np.asarray(inputs["encoder_key_padding_mask"]).astype(f)
    g1, b1 = np.asarray(inputs["g1"], f), np.asarray(inputs["b1"], f)
    g2, b2 = np.asarray(inputs["g2"], f), np.asarray(inputs["b2"], f)
    g3, b3 = np.asarray(inputs["g3"], f), np.asarray(inputs["b3"], f)

    def fold(g, W):
        return np.ascontiguousarray(g[:, None] * np.asarray(W, f))

    Wq1 = fold(g1, inputs["Wq1"])
    Wk1 = fold(g1, inputs["Wk1"])
    Wv1 = fold(g1, inputs["Wv1"])
    Wo1 = np.ascontiguousarray(np.asarray(inputs["Wo1"], f))
    qb1 = np.ascontiguousarray(b1 @ np.asarray(inputs["Wq1"], f))
    kb1 = np.ascontiguousarray(b1 @ np.asarray(inputs["Wk1"], f))
    vb1 = np.ascontiguousarray(b1 @ np.asarray(inputs["Wv1"], f))
    Wq2 = fold(g2, inputs["Wq2"])
    Wk2 = np.ascontiguousarray(np.asarray(inputs["Wk2"], f))
    Wv2 = np.ascontiguousarray(np.asarray(inputs["Wv2"], f))
    Wo2 = np.ascontiguousarray(np.asarray(inputs["Wo2"], f))
    qb2 = np.ascontiguousarray(b2 @ np.asarray(inputs["Wq2"], f))
    Wff1 = fold(g3, inputs["Wff1"])
    bff1 = np.ascontiguousarray(
        np.asarray(inputs["bff1"], f) + b3 @ np.asarray(inputs["Wff1"], f))
    Wff2 = np.ascontiguousarray(np.asarray(inputs["Wff2"], f))

    shared = {
        "Wq1": Wq1, "Wk1": Wk1, "Wv1": Wv1, "Wo1": Wo1,
        "qb1": qb1, "kb1": kb1, "vb1": vb1,
        "bo1": np.ascontiguousarray(np.asarray(inputs["bo1"], f)),
        "Wq2": Wq2, "Wk2": Wk2, "Wv2": Wv2, "Wo2": Wo2,
        "qb2": qb2,
        "bo2": np.ascontiguousarray(np.asarray(inputs["bo2"], f)),
        "Wff1": Wff1, "bff1": bff1, "Wff2": Wff2,
        "bff2": np.ascontiguousarray(np.asarray(inputs["bff2"], f)),
    }

    in_maps = []
    for core in range(NCORES):
        b, q = core // 2, core % 2
        x = hidden[b] if q == 0 else np.roll(hidden[b], -SQ, axis=0)
        in_maps.append({
            **shared,
            "x": np.ascontiguousarray(x),
            "ctxT": np.ascontiguousarray(context[b].T),
            "mask_f": np.ascontiguousarray(mask[b]),
        })
    return in_maps


def run(inputs, **spmd_kwargs):
    res = run_bass_kernel_spmd(
        _get_nc(), _make_in_maps(inputs), core_ids=list(range(NCORES)),
        **spmd_kwargs)
    out = np.empty((B, S, D), np.float32)
    for core in range(NCORES):
        b, q = core // 2, core % 2
        out[b, q * SQ:(q + 1) * SQ] = res.results[core]["out"]
    return out, res


def kernel(**inputs):
    out, _ = run(inputs)
    return out



# revision 29
# speedup vs baseline: 1.4130x; 1.4130x over previous
"""BasicTransformerBlock (self-attn + cross-attn + GEGLU FF) on 8 TRN2 cores.

Sharding: sequence-parallel, no collectives. B=4 batches x 2 sequence-halves
= 8 shards; each core computes 512 query rows end-to-end, duplicating only
the K/V projections for its batch. The host rolls each batch's hidden_states
so a core's query rows are always rows 0..511 - the kernel is uniform SPMD.

v2: all matmuls in fp8 e4m3 with DoubleRow perf mode (0.5 cyc/row, 256-deep
contraction per instruction). Fixed power-of-2 scale conventions:
  - weights quantized x16 host-side (x64 for Wff2)
  - activations (xn) quantized at true scale; q8/k8 carry 16x and the
    exp scale absorbs 1/256
  - V columns carry 16x, fused denominator column = 0.25 so the normalize
    step yields 64x attention output (good fp8 range)
  - attn(64x) @ Wo(16x) -> PSUM carries 1024x; the residual buffer hbuf
    holds 1024*h throughout (LayerNorm is scale-invariant); the host
    divides the final output by 1024 (exact)
  - FF: hT=16h', gT=gelu(g) true, mT=16m, Wff2 x64 -> PSUM 1024x
Scores for head h contract over d=64 as [32 partitions x 2 DoubleRow
k-tiles]; Wq/Wk columns are permuted host-side so the projections emit q/k
directly in that layout. Transposes run on PE in bf16 (fp8 transpose is
rejected by the walrus verifier); the PSUM->SBUF copy converts to fp8.

Engines: ACT = exp + gelu + q1/k1 evac; DVE = LN, PSUM evacs, PV normalize;
Pool (gpsimd, SBUF-only) = half the ff multiplies, hbuf scaling, fills.
Program order pipelines per 128-row query chunk (sc) so ACT stays saturated
with exp through both attention blocks.
"""

import contextlib
import os

import numpy as np
import ml_dtypes

_KSTOP = int(os.environ.get("KSTOP", "99"))

import concourse.mybir as mybir
import concourse.tile as tile
from concourse import bacc
from concourse.bass_utils import run_bass_kernel_spmd
from concourse.masks import make_identity

P = 128
B, S, T, D, H, DH = 4, 1024, 1024, 1024, 16, 64
FF = 4 * D
SQ = 512                 # query rows per core
SCALE = DH ** -0.5
EPS = 1e-12
NCORES = 8

f32 = mybir.dt.float32
bf16 = mybir.dt.bfloat16
fp8 = mybir.dt.float8e4
AF = mybir.ActivationFunctionType
ALU = mybir.AluOpType
DR = mybir.MatmulPerfMode.DoubleRow

DSUB = D // P            # 8
TSUB = T // P            # 8
SSUB = S // P            # 8
QSUB = SQ // P           # 4
FSUB = FF // P           # 32

SW = 16.0                # weight quant scale (x64 for Wff2)
SEXP = SCALE / 256.0     # exp scale: q,k both carry 16x
C64 = 0.25               # fused denominator column value -> attn_nat = 64x
SH = 1024.0              # hbuf carries 1024*h

FP8T = ml_dtypes.float8_e4m3fn


# --------------------------------------------------------------------------
# device-program helpers
# --------------------------------------------------------------------------

def _ln_stats(nc, sb_small, x_ap, eps_ap):
    """Return (mv, rstd): per-row mean/var and 1/sqrt(var+eps)."""
    stats = sb_small.tile([P, D // 512, 6], f32, tag="ln_stats")
    for c in range(D // 512):
        nc.vector.bn_stats(stats[:, c], x_ap[:, c * 512:(c + 1) * 512])
    mv = sb_small.tile([P, 2], f32, tag="ln_mv")
    nc.vector.bn_aggr(mv, stats)
    std = sb_small.tile([P, 1], f32, tag="ln_std")
    nc.scalar.activation(std, mv[:, 1:2], AF.Sqrt, bias=eps_ap)
    rstd = sb_small.tile([P, 1], f32, tag="ln_rstd")
    nc.vector.reciprocal(rstd, std)
    return mv, rstd


def _ln_norm(nc, x_ap, xn_ap, mv, rstd):
    for c in range(2):
        sl = slice(c * (D // 2), (c + 1) * (D // 2))
        nc.vector.tensor_scalar(xn_ap[:, sl], x_ap[:, sl], mv[:, 0:1], rstd,
                                ALU.subtract, ALU.mult)


def _transpose8(nc, tr_ps, identb, src_nat, dst8):
    """PE-transpose src_nat [P, D] bf16 into dst8 [P, 8, 128] fp8."""
    for half in range(2):
        tp = tr_ps.tile([P, 4, P], bf16, tag="tr")
        for i in range(4):
            dsb = half * 4 + i
            nc.tensor.transpose(
                tp[:, i], src_nat[:, dsb * P:(dsb + 1) * P], identb)
        nc.vector.tensor_copy(dst8[:, half * 4:(half + 1) * 4, :], tp)


def _proj_dr(nc, ps, w8, rhs8, j, ncols, col0=0, first=True, last=True):
    """ps[128, ncols] (+)= (W block j).T @ rhs over D=1024 (4 DoubleRow mms)."""
    for ksp in range(4):
        nc.tensor.matmul(
            ps,
            w8[:, 2 * ksp:2 * ksp + 2, j * P:(j + 1) * P],
            rhs8[:, 2 * ksp:2 * ksp + 2, col0:col0 + ncols],
            start=(first and ksp == 0), stop=(last and ksp == 3),
            perf_mode=DR)


def _attn_heads(nc, pools, k8, v8, q8_slices, attn_nat, sc, interleave):
    """16 heads: scoresT -> exp -> PV(fused denom) -> normalize (per 8)."""
    sc_pool, et_pool, pv_pool, sb_small = pools
    pv = None
    for h in range(H):
        a, r = h // 4, h % 4
        p0 = r * 32
        ps_sc = sc_pool.tile([P, TSUB, P], f32, tag="sc")
        for tb in range(TSUB):
            nc.tensor.matmul(
                ps_sc[:, tb],
                k8[p0:p0 + 32, a, :, tb * P:(tb + 1) * P],
                q8_slices(p0, a, sc),
                start=True, stop=True, perf_mode=DR,
                tile_position=(p0, 0))
        ET = et_pool.tile([P, TSUB, P], fp8, tag="ET")
        nc.scalar.activation(ET, ps_sc, AF.Exp, scale=SEXP)
        if h % 8 == 0:
            pv = pv_pool.tile([P, 8, P], f32, tag="pv")
        for tsp in range(4):
            nc.tensor.matmul(
                pv[:, h % 8, 0:65],
                ET[:, 2 * tsp:2 * tsp + 2, :],
                v8[:, 2 * tsp:2 * tsp + 2, h, :],
                start=(tsp == 0), stop=(tsp == 3), perf_mode=DR)
        if h % 8 == 7:
            hh = h // 8
            rec = sb_small.tile([P, 8], f32, tag="pv_rec")
            nc.vector.reciprocal(rec, pv[:, :, 64])
            nc.vector.tensor_tensor(
                attn_nat[:, hh * 512:(hh + 1) * 512],
                pv[:, :, 0:64],
                rec[:, :, None].to_broadcast((P, 8, 64)),
                ALU.mult)
        interleave(h)


# --------------------------------------------------------------------------
# full program
# --------------------------------------------------------------------------

def build_nc(with_vb1=False, with_bo1=False, with_bo2=False, with_bff2=False,
             with_qb1=False, with_kb1=False, with_qb2=False):
    nc = bacc.Bacc(None, target_bir_lowering=False, debug=False)

    x_dr = nc.dram_tensor("x", [S, D], f32, kind="ExternalInput")
    ctxT_dr = nc.dram_tensor("ctxT8", [D, T], fp8, kind="ExternalInput")
    ctxTm_dr = nc.dram_tensor("ctxTm8", [D, T], fp8, kind="ExternalInput")
    mcol_dr = nc.dram_tensor("mcol", [T], f32, kind="ExternalInput")
    wdr = {}
    for nm in ("Wq1", "Wk1", "Wv1", "Wo1", "Wq2", "Wk2", "Wv2", "Wo2"):
        wdr[nm] = nc.dram_tensor(nm, [D, D], fp8, kind="ExternalInput")
    wff1_dr = nc.dram_tensor("Wff1", [D, 2 * FF], fp8, kind="ExternalInput")
    wff1l_dr = nc.dram_tensor("Wff1l", [D, 2 * FF], fp8, kind="ExternalInput")
    wff1h_dr = nc.dram_tensor("Wff1h", [D, 2 * FF], fp8, kind="ExternalInput")
    wff2_dr = nc.dram_tensor("Wff2", [FF, D], fp8, kind="ExternalInput")
    qb1_dr = nc.dram_tensor("qb1_16", [D], f32, kind="ExternalInput")
    kb1_dr = nc.dram_tensor("kb1_16", [D], f32, kind="ExternalInput")
    qb2_dr = nc.dram_tensor("qb2_16", [D], f32, kind="ExternalInput")
    vb1_dr = nc.dram_tensor("vb1_16", [D], f32, kind="ExternalInput")
    bo1_dr = nc.dram_tensor("bo1_s", [D], f32, kind="ExternalInput")
    bo2_dr = nc.dram_tensor("bo2_s", [D], f32, kind="ExternalInput")
    bff2_dr = nc.dram_tensor("bff2_s", [D], f32, kind="ExternalInput")
    bffh_dr = nc.dram_tensor("bffh_16", [FF], f32, kind="ExternalInput")
    bffg_dr = nc.dram_tensor("bffg", [FF], f32, kind="ExternalInput")
    out_dr = nc.dram_tensor("out", [SQ, D], f32, kind="ExternalOutput")

    x_tiled = x_dr.rearrange("(ss p) d -> p ss d", p=P)
    w_t = {k: v.rearrange("(ks p) o -> p ks o", p=P) for k, v in wdr.items()}

    with tile.TileContext(nc) as tc, contextlib.ExitStack() as es:
        const = es.enter_context(tc.tile_pool(name="const", bufs=1))
        sb_small = es.enter_context(tc.tile_pool(name="smalls", bufs=8))

        identb = const.tile([P, P], bf16)
        make_identity(nc, identb)
        eps_ap = const.tile([P, 1], f32)
        nc.vector.memset(eps_ap, EPS)
        c1024 = const.tile([P, 1], f32)
        nc.vector.memset(c1024, SH)
        c16 = const.tile([P, 1], f32)
        nc.vector.memset(c16, 16.0)
        cinv16 = const.tile([P, 1], f32)
        nc.vector.memset(cinv16, 1.0 / 16.0)
        qb1_s = const.tile([P, DSUB], f32)
        nc.sync.dma_start(qb1_s, qb1_dr.rearrange("(c p) -> p c", p=P))
        kb1_s = const.tile([P, DSUB], f32)
        nc.sync.dma_start(kb1_s, kb1_dr.rearrange("(c p) -> p c", p=P))
        qb2_s = const.tile([P, DSUB], f32)
        nc.sync.dma_start(qb2_s, qb2_dr.rearrange("(c p) -> p c", p=P))
        bffh_s = const.tile([P, FSUB], f32)
        nc.sync.dma_start(bffh_s, bffh_dr.rearrange("(c p) -> p c", p=P))
        bffg_s = const.tile([P, FSUB], f32)
        nc.sync.dma_start(bffg_s, bffg_dr.rearrange("(c p) -> p c", p=P))
        mcol_s = const.tile([P, TSUB], f32)
        nc.sync.dma_start(mcol_s, mcol_dr.rearrange("(c p) -> p c", p=P))
        bo1_b = bo2_b = bff2_b = vb1_b = None
        if with_vb1:
            vb1_b = const.tile([P, D], f32)
            nc.sync.dma_start(vb1_b, vb1_dr[None, :].to_broadcast((P, D)))
        if with_bo1:
            bo1_b = const.tile([P, D], f32)
            nc.sync.dma_start(bo1_b, bo1_dr[None, :].to_broadcast((P, D)))
        if with_bo2:
            bo2_b = const.tile([P, D], f32)
            nc.sync.dma_start(bo2_b, bo2_dr[None, :].to_broadcast((P, D)))
        if with_bff2:
            bff2_b = const.tile([P, D], f32)
            nc.sync.dma_start(bff2_b, bff2_dr[None, :].to_broadcast((P, D)))

        # ---------- persistent tiles (freed in LIFO order) ----------
        hbuf, free_hbuf = tc.tile([P, QSUB, D], f32, name="hbuf")
        xn3T8, free_xn3T8 = tc.tile([P, DSUB, SQ], fp8, name="xn3T8")
        xn3loT8, free_xn3loT8 = tc.tile([P, DSUB, SQ], fp8, name="xn3loT8")
        q1_8, free_q1 = tc.tile([P, 4, 2, SQ], fp8, name="q1_8")
        k1_8, free_k1 = tc.tile([P, 4, 2, T], fp8, name="k1_8")
        v1_8, free_v1 = tc.tile([P, TSUB, H, 65], fp8, name="v1_8")
        k2_8, free_k2 = tc.tile([P, 4, 2, T], fp8, name="k2_8")
        v2_8, free_v2 = tc.tile([P, TSUB, H, 65], fp8, name="v2_8")

        # ---------- preamble: LN1 over full S + transposes ----------
        w1_es = contextlib.ExitStack()
        wpool1 = w1_es.enter_context(tc.tile_pool(name="w1", bufs=3))
        wq1_sb = wpool1.tile([P, DSUB, D], fp8, tag="wq1")
        nc.sync.dma_start(wq1_sb, w_t["Wq1"])
        wk1_sb = wpool1.tile([P, DSUB, D], fp8, tag="wk1")
        nc.sync.dma_start(wk1_sb, w_t["Wk1"])
        wv1_sb = wpool1.tile([P, DSUB, D], fp8, tag="wv1")
        nc.sync.dma_start(wv1_sb, w_t["Wv1"])
        xn1T8, free_xn1T8 = tc.tile([P, DSUB, S], fp8, name="xn1T8")

        if _KSTOP >= 1:
         with (
            nc.named_scope("ln1"),
            tc.tile_pool(name="x_in", bufs=3) as x_pool,
            tc.tile_pool(name="xn1", bufs=3) as xn1_pool,
            tc.tile_pool(name="tr1_ps", bufs=4, space="PSUM") as tr1_ps,
         ):
            for ss in range(SSUB):
                xt = x_pool.tile([P, D], f32, tag="x")
                nc.sync.dma_start(xt, x_tiled[:, ss])
                mv, rstd = _ln_stats(nc, sb_small, xt, eps_ap)
                xn = xn1_pool.tile([P, D], bf16, tag="xn1")
                _ln_norm(nc, xt, xn, mv, rstd)
                if ss < QSUB:
                    nc.gpsimd.tensor_scalar(
                        hbuf[:, ss], xt, c1024, None, ALU.mult)
                    if with_bo1:
                        nc.gpsimd.tensor_tensor(
                            hbuf[:, ss], hbuf[:, ss], bo1_b, ALU.add)
                _transpose8(nc, tr1_ps, identb, xn,
                            xn1T8[:, :, ss * P:(ss + 1) * P])

        # ---------- preamble: QKV1 ----------
        if _KSTOP >= 2:
         with (
            nc.named_scope("qkv1"),
            tc.tile_pool(name="qkv1_ps", bufs=4, space="PSUM") as ps_proj,
         ):
            for j in range(DSUB):
                ps = ps_proj.tile([P, 512], f32, tag="proj")
                _proj_dr(nc, ps, wq1_sb, xn1T8, j, SQ)
                if with_qb1:
                    nc.scalar.activation(
                        q1_8[:, j // 2, j % 2, :], ps, AF.Identity,
                        bias=qb1_s[:, j:j + 1])
                else:
                    nc.scalar.copy(q1_8[:, j // 2, j % 2, :], ps)
            for j in range(DSUB):
                for tch in range(2):
                    ps = ps_proj.tile([P, 512], f32, tag="proj")
                    _proj_dr(nc, ps, wk1_sb, xn1T8, j, 512, col0=tch * 512)
                    dst = k1_8[:, j // 2, j % 2, tch * 512:(tch + 1) * 512]
                    if with_kb1:
                        nc.scalar.activation(
                            dst, ps, AF.Identity, bias=kb1_s[:, j:j + 1])
                    else:
                        nc.scalar.copy(dst, ps)
            for ts in range(TSUB):
                for dh in range(2):
                    ps = ps_proj.tile([P, 512], f32, tag="proj")
                    for ksp in range(4):
                        nc.tensor.matmul(
                            ps,
                            xn1T8[:, 2 * ksp:2 * ksp + 2, ts * P:(ts + 1) * P],
                            wv1_sb[:, 2 * ksp:2 * ksp + 2,
                                   dh * 512:(dh + 1) * 512],
                            start=(ksp == 0), stop=(ksp == 3), perf_mode=DR)
                    dst = v1_8[:, ts, dh * 8:(dh + 1) * 8, 0:64]
                    src = ps.rearrange("p (h w) -> p h w", h=8)
                    if with_vb1:
                        nc.vector.tensor_tensor(
                            dst, src,
                            vb1_b[:, dh * 512:(dh + 1) * 512].rearrange(
                                "p (h w) -> p h w", h=8),
                            ALU.add)
                    else:
                        nc.vector.tensor_copy(dst, src)
            nc.gpsimd.memset(v1_8[:, :, :, 64:65], C64)
        free_xn1T8()
        w1_es.close()

        # weights/inputs needed during the main loop
        wo1_sb, free_wo1 = tc.tile([P, DSUB, D], fp8, name="wo1")
        nc.sync.dma_start(wo1_sb, w_t["Wo1"])
        wq2_sb, free_wq2 = tc.tile([P, DSUB, D], fp8, name="wq2")
        nc.sync.dma_start(wq2_sb, w_t["Wq2"])
        wo2_sb, free_wo2 = tc.tile([P, DSUB, D], fp8, name="wo2")
        nc.sync.dma_start(wo2_sb, w_t["Wo2"])
        ctx_es = contextlib.ExitStack()
        ctxpool = ctx_es.enter_context(tc.tile_pool(name="ctx", bufs=1))
        ctxT_sb = ctxpool.tile([P, DSUB, T], fp8, tag="ctxT")
        nc.sync.dma_start(ctxT_sb, ctxT_dr.rearrange("(ds p) t -> p ds t", p=P))
        ctxTm_sb = ctxpool.tile([P, DSUB, T], fp8, tag="ctxTm")
        nc.sync.dma_start(ctxTm_sb, ctxTm_dr.rearrange("(ds p) t -> p ds t", p=P))
        wk2_sb = ctxpool.tile([P, DSUB, D], fp8, tag="wk2")
        nc.sync.dma_start(wk2_sb, w_t["Wk2"])
        wv2_sb = ctxpool.tile([P, DSUB, D], fp8, tag="wv2")
        nc.sync.dma_start(wv2_sb, w_t["Wv2"])

        # ---------- main per-sc loop ----------
        main_es = contextlib.ExitStack()
        sc_pool = main_es.enter_context(
            tc.tile_pool(name="scores", bufs=2, space="PSUM"))
        pv_pool = main_es.enter_context(
            tc.tile_pool(name="pv", bufs=1, space="PSUM"))
        tr_ps = main_es.enter_context(
            tc.tile_pool(name="tr_ps", bufs=1, space="PSUM"))
        proj_ps = main_es.enter_context(
            tc.tile_pool(name="proj_ps", bufs=1, space="PSUM"))
        et_pool = main_es.enter_context(tc.tile_pool(name="ET", bufs=3))
        nat_pool = main_es.enter_context(tc.tile_pool(name="nat", bufs=2))
        aT_pool = main_es.enter_context(tc.tile_pool(name="aT", bufs=2))
        q2_pool = main_es.enter_context(tc.tile_pool(name="q2", bufs=2))

        # k2/v2 chunk emitters, interleaved into attn1 of sc 0/1
        k2v2_chunks = []

        def _k2_chunk(j, tch):
            def emit():
                ps = proj_ps.tile([P, 512], f32, tag="mps")
                _proj_dr(nc, ps, wk2_sb, ctxT_sb, j, 512, col0=tch * 512)
                nc.vector.tensor_copy(
                    k2_8[:, j // 2, j % 2, tch * 512:(tch + 1) * 512], ps)
            return emit

        def _v2_chunk(ts, dh):
            def emit():
                ps = proj_ps.tile([P, 512], f32, tag="mps")
                for ksp in range(4):
                    nc.tensor.matmul(
                        ps,
                        ctxTm_sb[:, 2 * ksp:2 * ksp + 2, ts * P:(ts + 1) * P],
                        wv2_sb[:, 2 * ksp:2 * ksp + 2, dh * 512:(dh + 1) * 512],
                        start=(ksp == 0), stop=(ksp == 3), perf_mode=DR)
                nc.vector.tensor_copy(
                    v2_8[:, ts, dh * 8:(dh + 1) * 8, 0:64],
                    ps.rearrange("p (h w) -> p h w", h=8))
                if dh == 1:
                    nc.gpsimd.tensor_copy(
                        v2_8[:, ts, :, 64],
                        mcol_s[:, ts:ts + 1].to_broadcast((P, H)))
            return emit

        for j in range(DSUB):
            for tch in range(2):
                k2v2_chunks.append(_k2_chunk(j, tch))
        for ts in range(TSUB):
            for dh in range(2):
                k2v2_chunks.append(_v2_chunk(ts, dh))
        k2v2_pos = [0]

        def interleave(h):
            # All 32 chunks must be emitted before attn2(sc0) is emitted:
            # 2 per head over attn1(sc0)'s 16 heads.
            budget = 2
            while budget and k2v2_pos[0] < len(k2v2_chunks):
                k2v2_chunks[k2v2_pos[0]]()
                k2v2_pos[0] += 1
                budget -= 1

        def no_interleave(h):
            pass

        attn_pools = (sc_pool, et_pool, pv_pool, sb_small)

        def q1_slices(p0, a, sc):
            return q1_8[p0:p0 + 32, a, :, sc * P:(sc + 1) * P]

        def _wo_block(sc, attn_nat, wo_sb, bias_b):
            aT = aT_pool.tile([P, DSUB, P], fp8, tag="aT")
            _transpose8(nc, tr_ps, identb, attn_nat, aT)
            for dh in range(2):
                ps = proj_ps.tile([P, 512], f32, tag="mps")
                for ksp in range(4):
                    nc.tensor.matmul(
                        ps,
                        aT[:, 2 * ksp:2 * ksp + 2, :],
                        wo_sb[:, 2 * ksp:2 * ksp + 2, dh * 512:(dh + 1) * 512],
                        start=(ksp == 0), stop=(ksp == 3), perf_mode=DR)
                sl = slice(dh * 512, (dh + 1) * 512)
                nc.vector.tensor_tensor(
                    hbuf[:, sc, sl], ps, hbuf[:, sc, sl], ALU.add)
            if bias_b is not None:
                nc.gpsimd.tensor_tensor(
                    hbuf[:, sc], hbuf[:, sc], bias_b, ALU.add)

        def _ln_block(sc, xnT8_dst, xlo_dst=None):
            mv, rstd = _ln_stats(nc, sb_small, hbuf[:, sc], eps_ap)
            xn = nat_pool.tile([P, D], bf16, tag="xn")
            _ln_norm(nc, hbuf[:, sc], xn, mv, rstd)
            for half in range(2):
                tp = tr_ps.tile([P, 4, P], bf16, tag="tr")
                for i in range(4):
                    dsb = half * 4 + i
                    nc.tensor.transpose(
                        tp[:, i], xn[:, dsb * P:(dsb + 1) * P], identb)
                dst = xnT8_dst[:, half * 4:(half + 1) * 4, :]
                nc.vector.tensor_copy(dst, tp)
                if xlo_dst is not None:
                    # fp8 residual of the quantization, scaled x16
                    t = nat_pool.tile([P, 4, P], bf16, tag="xlo_t")
                    nc.vector.tensor_tensor(t, tp, dst, ALU.subtract)
                    nc.vector.tensor_scalar(
                        xlo_dst[:, half * 4:(half + 1) * 4, :], t, c16, None,
                        ALU.mult)

        a1_nats = {}

        def _block2(sc):
            """wo1 -> ln2 -> q2 -> attn2 -> wo2 -> ln3 for one sc chunk."""
            if _KSTOP >= 4:
             with nc.named_scope("wo1"):
                _wo_block(sc, a1_nats.pop(sc), wo1_sb, bo1_b)
            if _KSTOP >= 5:
             with nc.named_scope("ln2"):
                xn2T8 = aT_pool.tile([P, DSUB, P], fp8, tag="xn2T")
                _ln_block(sc, xn2T8)
             with nc.named_scope("q2"):
                q2t = q2_pool.tile([P, 4, 2, P], fp8, tag="q2")
                for jp in range(2):
                    ps = proj_ps.tile([P, 512], f32, tag="mps")
                    psv = ps.rearrange("p (i s) -> p i s", i=4)
                    for i in range(4):
                        _proj_dr(nc, psv[:, i], wq2_sb, xn2T8, jp * 4 + i, P)
                    dst = q2t[:, 2 * jp:2 * jp + 2, :, :]
                    src = ps.rearrange("p (a k s) -> p a k s", a=2, k=2)
                    if with_qb2:
                        nc.vector.tensor_tensor(
                            dst, src,
                            qb2_s[:, 4 * jp:4 * jp + 4].rearrange(
                                "p (a k) -> p a k", a=2
                            )[:, :, :, None].to_broadcast((P, 2, 2, P)),
                            ALU.add)
                    else:
                        nc.vector.tensor_copy(dst, src)
            if _KSTOP >= 6:
             with nc.named_scope("attn2"):
                attn2_nat = nat_pool.tile([P, D], bf16, tag="a2nat")

                def q2_slices(p0, a, _sc, q2t=q2t):
                    return q2t[p0:p0 + 32, a, :, :]

                _attn_heads(nc, attn_pools, k2_8, v2_8, q2_slices, attn2_nat,
                            sc, no_interleave)
            if _KSTOP >= 7:
             with nc.named_scope("wo2"):
                _wo_block(sc, attn2_nat, wo2_sb, bo2_b)
            if _KSTOP >= 8:
             with nc.named_scope("ln3"):
                _ln_block(sc, xn3T8[:, :, sc * P:(sc + 1) * P],
                          xn3loT8[:, :, sc * P:(sc + 1) * P])

        # Skewed pipeline: block2(sc-1) is emitted after attn1(sc), so the
        # wo1->ln2->q2 chain of sc-1 resolves while ACT runs exp1(sc).
        for sc in range(QSUB):
            if _KSTOP >= 3:
             with nc.named_scope("attn1"):
                a1_nats[sc] = nat_pool.tile([P, D], bf16, tag="a1nat",
                                            name=f"a1nat{sc}")
                _attn_heads(nc, attn_pools, k1_8, v1_8, q1_slices,
                            a1_nats[sc], sc,
                            interleave if sc == 0 else no_interleave)
            if sc >= 1:
                _block2(sc - 1)
        if _KSTOP >= 3:
            _block2(QSUB - 1)

        main_es.close()
        ctx_es.close()
        free_wo2(); free_wq2(); free_wo1()
        free_v2(); free_k2(); free_v1(); free_k1(); free_q1()

        # ---------- GEGLU feed-forward ----------
        # FF1 PSUM carries 256*(xn@Wff1) via three accumulated fp8 terms:
        # xn8 @ (16*Whi) + xn8 @ Wlo + xlo8 @ Whi, with Wlo = fp8 residual
        # of 16*Wff1 scaled x16 and xlo8 the x16 fp8 residual of xn.
        mT8, free_mT8 = tc.tile([P, FSUB, SQ], fp8, name="mT8")
        wff2_sb, free_wff2 = tc.tile([P, FSUB, D], fp8, name="wff2")
        nc.sync.dma_start(wff2_sb, wff2_dr.rearrange("(ks p) o -> p ks o", p=P))
        wff1_t = wff1_dr.rearrange("(ks p) f -> p ks f", p=P)
        wff1l_t = wff1l_dr.rearrange("(ks p) f -> p ks f", p=P)
        wff1h_t = wff1h_dr.rearrange("(ks p) f -> p ks f", p=P)
        if _KSTOP >= 9:
         with (
            nc.named_scope("ff1"),
            tc.tile_pool(name="wff1", bufs=2) as wff1_pool,
            tc.tile_pool(name="ff1_ps", bufs=2, space="PSUM") as ps_ff1,
            tc.tile_pool(name="hT", bufs=3) as hT_pool,
            tc.tile_pool(name="gT", bufs=3) as gT_pool,
         ):
            def _load3(c0, sfx):
                w16 = wff1_pool.tile([P, DSUB, D], fp8, tag="w16" + sfx)
                nc.sync.dma_start(w16, wff1_t[:, :, c0:c0 + D])
                wlo = wff1_pool.tile([P, DSUB, D], fp8, tag="wlo" + sfx)
                nc.sync.dma_start(wlo, wff1l_t[:, :, c0:c0 + D])
                whi = wff1_pool.tile([P, DSUB, D], fp8, tag="whi" + sfx)
                nc.sync.dma_start(whi, wff1h_t[:, :, c0:c0 + D])
                return w16, wlo, whi

            def _ff1_mm(ps, w3, fi, rhs_hi, rhs_lo):
                w16, wlo, whi = w3
                _proj_dr(nc, ps, w16, rhs_hi, fi, SQ, last=False)
                _proj_dr(nc, ps, wlo, rhs_hi, fi, SQ, first=False, last=False)
                _proj_dr(nc, ps, whi, rhs_lo, fi, SQ, first=False)

            for cc in range(4):
                wh3 = _load3(cc * D, "h")
                wg3 = _load3(FF + cc * D, "g")
                for fi in range(DSUB):
                    fc = cc * DSUB + fi
                    ps_h = ps_ff1.tile([P, SQ], f32, tag="psh")
                    _ff1_mm(ps_h, wh3, fi, xn3T8, xn3loT8)
                    ps_g = ps_ff1.tile([P, SQ], f32, tag="psg")
                    _ff1_mm(ps_g, wg3, fi, xn3T8, xn3loT8)
                    hT = hT_pool.tile([P, SQ], bf16, tag="hT")
                    nc.vector.tensor_scalar(
                        hT, ps_h, cinv16, bffh_s[:, fc:fc + 1],
                        ALU.mult, ALU.add)
                    gT = gT_pool.tile([P, SQ], bf16, tag="gT")
                    nc.scalar.activation(
                        gT, ps_g, AF.Gelu, bias=bffg_s[:, fc:fc + 1],
                        scale=1.0 / (SW * 16.0))
                    eng = nc.gpsimd if fc % 2 else nc.vector
                    eng.tensor_tensor(mT8[:, fc, :], hT, gT, ALU.mult)

        if _KSTOP >= 10:
         with (
            nc.named_scope("ff2"),
            tc.tile_pool(name="ff2_ps", bufs=1, space="PSUM") as ps_ff2,
         ):
            ps_o = [ps_ff2.tile([P, 512], f32, tag=f"o{i}", name=f"ps_o{i}")
                    for i in range(8)]
            for ksp in range(FSUB // 2):
                for sc in range(QSUB):
                    for dh in range(2):
                        nc.tensor.matmul(
                            ps_o[sc * 2 + dh],
                            mT8[:, 2 * ksp:2 * ksp + 2, sc * P:(sc + 1) * P],
                            wff2_sb[:, 2 * ksp:2 * ksp + 2,
                                    dh * 512:(dh + 1) * 512],
                            start=(ksp == 0), stop=(ksp == FSUB // 2 - 1),
                            perf_mode=DR)
            for sc in range(QSUB):
                if with_bff2:
                    nc.gpsimd.tensor_tensor(
                        hbuf[:, sc], hbuf[:, sc], bff2_b, ALU.add)
                for dh in range(2):
                    sl = slice(dh * 512, (dh + 1) * 512)
                    nc.vector.tensor_tensor(
                        hbuf[:, sc, sl], ps_o[sc * 2 + dh],
                        hbuf[:, sc, sl], ALU.add)
        for sc in range(QSUB):
            nc.sync.dma_start(
                out_dr.rearrange("(ss p) d -> p ss d", p=P)[:, sc],
                hbuf[:, sc])
        free_wff2(); free_mT8()
        free_xn3loT8(); free_xn3T8(); free_hbuf()

    nc.compile()
    return nc


# --------------------------------------------------------------------------
# host side
# --------------------------------------------------------------------------

_NC = None
_NC_FLAGS = None


def _perm_qk():
    """Column permutation for Wq/Wk: head-interleaved DoubleRow layout."""
    perm = np.empty(D, np.int64)
    for h in range(H):
        for dl in range(DH):
            j = (h // 4) * 2 + (dl // 32)
            c = (h % 4) * 32 + (dl % 32)
            perm[j * P + c] = h * DH + dl
    return perm


_PERM = _perm_qk()


def _q8(x, scale=1.0):
    x = np.asarray(x, np.float32) * scale
    am = np.abs(x).max()
    assert am < 440, f"fp8 overflow: absmax {am}"
    return x.astype(FP8T)


def _get_nc(flags=()):
    global _NC, _NC_FLAGS
    flags = tuple(flags)
    if _NC is None or _NC_FLAGS != flags:
        _NC = build_nc(**dict(flags))
        _NC_FLAGS = flags
    return _NC


def _make_in_maps(inputs):
    f = np.float32
    hidden = np.asarray(inputs["hidden_states"], f)
    context = np.asarray(inputs["context"], f)
    mask = np.asarray(inputs["encoder_key_padding_mask"]).astype(f)
    g1, b1 = np.asarray(inputs["g1"], f), np.asarray(inputs["b1"], f)
    g2, b2 = np.asarray(inputs["g2"], f), np.asarray(inputs["b2"], f)
    g3, b3 = np.asarray(inputs["g3"], f), np.asarray(inputs["b3"], f)

    def fold(g, W):
        return g[:, None] * np.asarray(W, f)

    Wq1 = fold(g1, inputs["Wq1"])[:, _PERM]
    Wk1 = fold(g1, inputs["Wk1"])[:, _PERM]
    Wv1 = fold(g1, inputs["Wv1"])
    Wo1 = np.asarray(inputs["Wo1"], f)
    qb1 = (b1 @ np.asarray(inputs["Wq1"], f))[_PERM]
    kb1 = (b1 @ np.asarray(inputs["Wk1"], f))[_PERM]
    vb1 = b1 @ np.asarray(inputs["Wv1"], f)
    Wq2 = fold(g2, inputs["Wq2"])[:, _PERM]
    Wk2 = np.asarray(inputs["Wk2"], f)[:, _PERM]
    Wv2 = np.asarray(inputs["Wv2"], f)
    Wo2 = np.asarray(inputs["Wo2"], f)
    qb2 = (b2 @ np.asarray(inputs["Wq2"], f))[_PERM]
    Wff1 = fold(g3, inputs["Wff1"])
    bff1 = np.asarray(inputs["bff1"], f) + b3 @ np.asarray(inputs["Wff1"], f)
    Wff2 = np.asarray(inputs["Wff2"], f)
    bo1 = np.asarray(inputs["bo1"], f)
    bo2 = np.asarray(inputs["bo2"], f)
    bff2 = np.asarray(inputs["bff2"], f)

    flags = (
        ("with_vb1", bool(np.any(vb1))),
        ("with_bo1", bool(np.any(bo1))),
        ("with_bo2", bool(np.any(bo2))),
        ("with_bff2", bool(np.any(bff2))),
        ("with_qb1", bool(np.any(qb1))),
        ("with_kb1", bool(np.any(kb1))),
        ("with_qb2", bool(np.any(qb2))),
    )

    wff1_hi = _q8(Wff1, SW)
    wff1_hi_f = wff1_hi.astype(f)
    wff1_hi16 = _q8(wff1_hi_f, 16.0)          # exact exponent shift
    wff1_lo = _q8((SW * Wff1 - wff1_hi_f), 16.0)

    shared = {
        "Wq1": _q8(Wq1, SW), "Wk1": _q8(Wk1, SW), "Wv1": _q8(Wv1, SW),
        "Wo1": _q8(Wo1, SW),
        "Wq2": _q8(Wq2, SW), "Wk2": _q8(Wk2, SW), "Wv2": _q8(Wv2, SW),
        "Wo2": _q8(Wo2, SW),
        "Wff1": wff1_hi16, "Wff1l": wff1_lo, "Wff1h": wff1_hi,
        "Wff2": _q8(Wff2, 64.0),
        "qb1_16": np.ascontiguousarray(SW * qb1),
        "kb1_16": np.ascontiguousarray(SW * kb1),
        "qb2_16": np.ascontiguousarray(SW * qb2),
        "vb1_16": np.ascontiguousarray(SW * vb1),
        "bo1_s": np.ascontiguousarray(SH * bo1),
        "bo2_s": np.ascontiguousarray(SH * bo2),
        "bff2_s": np.ascontiguousarray(SH * bff2),
        "bffh_16": np.ascontiguousarray(SW * bff1[:FF]),
        "bffg": np.ascontiguousarray(bff1[FF:]),
    }

    in_maps = []
    for core in range(NCORES):
        b, q = core // 2, core % 2
        x = hidden[b] if q == 0 else np.roll(hidden[b], -SQ, axis=0)
        ctxT = np.ascontiguousarray(context[b].T)
        ctxTm = np.ascontiguousarray((mask[b][:, None] * context[b]).T)
        in_maps.append({
            **shared,
            "x": np.ascontiguousarray(x),
            "ctxT8": _q8(ctxT),
            "ctxTm8": _q8(ctxTm),
            "mcol": np.ascontiguousarray(C64 * mask[b]),
        })
    return in_maps, flags


def run(inputs, **spmd_kwargs):
    in_maps, flags = _make_in_maps(inputs)
    res = run_bass_kernel_spmd(
        _get_nc(flags), in_maps, core_ids=list(range(NCORES)),
        **spmd_kwargs)
    out = np.empty((B, S, D), np.float32)
    for core in range(NCORES):
        b, q = core // 2, core % 2
        out[b, q * SQ:(q + 1) * SQ] = res.results[core]["out"] * (1.0 / SH)
    return out, res


def kernel(**inputs):
    out, _ = run(inputs)
    return out


# revision 30
# speedup vs baseline: 1.4428x; 1.0211x over previous
"""BasicTransformerBlock (self-attn + cross-attn + GEGLU FF) on 8 TRN2 cores.

Sharding: sequence-parallel, no collectives. B=4 batches x 2 sequence-halves
= 8 shards; each core computes 512 query rows end-to-end, duplicating only
the K/V projections for its batch. The host rolls each batch's hidden_states
so a core's query rows are always rows 0..511 - the kernel is uniform SPMD.

v2: all matmuls in fp8 e4m3 with DoubleRow perf mode (0.5 cyc/row, 256-deep
contraction per instruction). Fixed power-of-2 scale conventions:
  - weights quantized x16 host-side (x64 for Wff2)
  - activations (xn) quantized at true scale; q8/k8 carry 16x and the
    exp scale absorbs 1/256
  - V columns carry 16x, fused denominator column = 0.25 so the normalize
    step yields 64x attention output (good fp8 range)
  - attn(64x) @ Wo(16x) -> PSUM carries 1024x; the residual buffer hbuf
    holds 1024*h throughout (LayerNorm is scale-invariant); the host
    divides the final output by 1024 (exact)
  - FF: hT=16h', gT=gelu(g) true, mT=16m, Wff2 x64 -> PSUM 1024x
Scores for head h contract over d=64 as [32 partitions x 2 DoubleRow
k-tiles]; Wq/Wk columns are permuted host-side so the projections emit q/k
directly in that layout. Transposes run on PE in bf16 (fp8 transpose is
rejected by the walrus verifier); the PSUM->SBUF copy converts to fp8.

Engines: ACT = exp + gelu + q1/k1 evac; DVE = LN, PSUM evacs, PV normalize;
Pool (gpsimd, SBUF-only) = half the ff multiplies, hbuf scaling, fills.
Program order pipelines per 128-row query chunk (sc) so ACT stays saturated
with exp through both attention blocks.
"""

import contextlib
import os

import numpy as np
import ml_dtypes

_KSTOP = int(os.environ.get("KSTOP", "99"))

import concourse.mybir as mybir
import concourse.tile as tile
from concourse import bacc
from concourse.bass_utils import run_bass_kernel_spmd
from concourse.masks import make_identity

P = 128
B, S, T, D, H, DH = 4, 1024, 1024, 1024, 16, 64
FF = 4 * D
SQ = 512                 # query rows per core
SCALE = DH ** -0.5
EPS = 1e-12
NCORES = 8

f32 = mybir.dt.float32
bf16 = mybir.dt.bfloat16
fp8 = mybir.dt.float8e4
AF = mybir.ActivationFunctionType
ALU = mybir.AluOpType
DR = mybir.MatmulPerfMode.DoubleRow

DSUB = D // P            # 8
TSUB = T // P            # 8
SSUB = S // P            # 8
QSUB = SQ // P           # 4
FSUB = FF // P           # 32

SW = 16.0                # weight quant scale (x64 for Wff2)
SEXP = SCALE / 256.0     # exp scale: q,k both carry 16x
C64 = 0.25               # fused denominator column value -> attn_nat = 64x
SH = 1024.0              # hbuf carries 1024*h

FP8T = ml_dtypes.float8_e4m3fn


# --------------------------------------------------------------------------
# device-program helpers
# --------------------------------------------------------------------------

def _ln_stats(nc, sb_small, x_ap, eps_ap):
    """Return (mv, rstd): per-row mean/var and 1/sqrt(var+eps)."""
    stats = sb_small.tile([P, D // 512, 6], f32, tag="ln_stats")
    for c in range(D // 512):
        nc.vector.bn_stats(stats[:, c], x_ap[:, c * 512:(c + 1) * 512])
    mv = sb_small.tile([P, 2], f32, tag="ln_mv")
    nc.vector.bn_aggr(mv, stats)
    std = sb_small.tile([P, 1], f32, tag="ln_std")
    nc.scalar.activation(std, mv[:, 1:2], AF.Sqrt, bias=eps_ap)
    rstd = sb_small.tile([P, 1], f32, tag="ln_rstd")
    nc.vector.reciprocal(rstd, std)
    return mv, rstd


def _ln_norm(nc, x_ap, xn_ap, mv, rstd):
    for c in range(2):
        sl = slice(c * (D // 2), (c + 1) * (D // 2))
        nc.vector.tensor_scalar(xn_ap[:, sl], x_ap[:, sl], mv[:, 0:1], rstd,
                                ALU.subtract, ALU.mult)


def _transpose8(nc, tr_ps, identb, src_nat, dst8):
    """PE-transpose src_nat [P, D] bf16 into dst8 [P, 8, 128] fp8."""
    for half in range(2):
        tp = tr_ps.tile([P, 4, P], bf16, tag="tr")
        for i in range(4):
            dsb = half * 4 + i
            nc.tensor.transpose(
                tp[:, i], src_nat[:, dsb * P:(dsb + 1) * P], identb)
        nc.vector.tensor_copy(dst8[:, half * 4:(half + 1) * 4, :], tp)


def _proj_dr(nc, ps, w8, rhs8, j, ncols, col0=0, first=True, last=True):
    """ps[128, ncols] (+)= (W block j).T @ rhs over D=1024 (4 DoubleRow mms)."""
    for ksp in range(4):
        nc.tensor.matmul(
            ps,
            w8[:, 2 * ksp:2 * ksp + 2, j * P:(j + 1) * P],
            rhs8[:, 2 * ksp:2 * ksp + 2, col0:col0 + ncols],
            start=(first and ksp == 0), stop=(last and ksp == 3),
            perf_mode=DR)


def _attn_heads(nc, pools, k8, v8, q8_slices, attn_nat, sc, interleave):
    """16 heads: scoresT -> exp -> PV(fused denom) -> normalize (per 8)."""
    sc_pool, et_pool, pv_pool, sb_small = pools
    pv = None
    for h in range(H):
        a, r = h // 4, h % 4
        p0 = r * 32
        ps_sc = sc_pool.tile([P, TSUB, P], f32, tag="sc")
        for tb in range(TSUB):
            nc.tensor.matmul(
                ps_sc[:, tb],
                k8[p0:p0 + 32, a, :, tb * P:(tb + 1) * P],
                q8_slices(p0, a, sc),
                start=True, stop=True, perf_mode=DR,
                tile_position=(p0, 0))
        ET = et_pool.tile([P, TSUB, P], fp8, tag="ET")
        nc.scalar.activation(ET, ps_sc, AF.Exp, scale=SEXP)
        if h % 8 == 0:
            pv = pv_pool.tile([P, 8, P], f32, tag="pv")
        for tsp in range(4):
            nc.tensor.matmul(
                pv[:, h % 8, 0:65],
                ET[:, 2 * tsp:2 * tsp + 2, :],
                v8[:, 2 * tsp:2 * tsp + 2, h, :],
                start=(tsp == 0), stop=(tsp == 3), perf_mode=DR)
        if h % 8 == 7:
            hh = h // 8
            rec = sb_small.tile([P, 8], f32, tag="pv_rec")
            nc.vector.reciprocal(rec, pv[:, :, 64])
            nc.vector.tensor_tensor(
                attn_nat[:, hh * 512:(hh + 1) * 512],
                pv[:, :, 0:64],
                rec[:, :, None].to_broadcast((P, 8, 64)),
                ALU.mult)
        interleave(h)


# --------------------------------------------------------------------------
# full program
# --------------------------------------------------------------------------

def build_nc(with_vb1=False, with_bo1=False, with_bo2=False, with_bff2=False,
             with_qb1=False, with_kb1=False, with_qb2=False):
    nc = bacc.Bacc(None, target_bir_lowering=False, debug=False)

    x_dr = nc.dram_tensor("x", [S, D], f32, kind="ExternalInput")
    ctxT_dr = nc.dram_tensor("ctxT8", [D, T], fp8, kind="ExternalInput")
    ctxTm_dr = nc.dram_tensor("ctxTm8", [D, T], fp8, kind="ExternalInput")
    mcol_dr = nc.dram_tensor("mcol", [T], f32, kind="ExternalInput")
    wdr = {}
    for nm in ("Wq1", "Wk1", "Wv1", "Wo1", "Wq2", "Wk2", "Wv2", "Wo2"):
        wdr[nm] = nc.dram_tensor(nm, [D, D], fp8, kind="ExternalInput")
    wff1_dr = nc.dram_tensor("Wff1", [D, 2 * FF], fp8, kind="ExternalInput")
    wff1l_dr = nc.dram_tensor("Wff1l", [D, 2 * FF], fp8, kind="ExternalInput")
    wff1h_dr = nc.dram_tensor("Wff1h", [D, 2 * FF], fp8, kind="ExternalInput")
    wff2_dr = nc.dram_tensor("Wff2", [FF, D], fp8, kind="ExternalInput")
    qb1_dr = nc.dram_tensor("qb1_16", [D], f32, kind="ExternalInput")
    kb1_dr = nc.dram_tensor("kb1_16", [D], f32, kind="ExternalInput")
    qb2_dr = nc.dram_tensor("qb2_16", [D], f32, kind="ExternalInput")
    vb1_dr = nc.dram_tensor("vb1_16", [D], f32, kind="ExternalInput")
    bo1_dr = nc.dram_tensor("bo1_s", [D], f32, kind="ExternalInput")
    bo2_dr = nc.dram_tensor("bo2_s", [D], f32, kind="ExternalInput")
    bff2_dr = nc.dram_tensor("bff2_s", [D], f32, kind="ExternalInput")
    bffh_dr = nc.dram_tensor("bffh_16", [FF], f32, kind="ExternalInput")
    bffg_dr = nc.dram_tensor("bffg", [FF], f32, kind="ExternalInput")
    out_dr = nc.dram_tensor("out", [SQ, D], f32, kind="ExternalOutput")

    x_tiled = x_dr.rearrange("(ss p) d -> p ss d", p=P)
    w_t = {k: v.rearrange("(ks p) o -> p ks o", p=P) for k, v in wdr.items()}

    with tile.TileContext(nc) as tc, contextlib.ExitStack() as es:
        const = es.enter_context(tc.tile_pool(name="const", bufs=1))
        sb_small = es.enter_context(tc.tile_pool(name="smalls", bufs=8))

        identb = const.tile([P, P], bf16)
        make_identity(nc, identb)
        eps_ap = const.tile([P, 1], f32)
        nc.vector.memset(eps_ap, EPS)
        c1024 = const.tile([P, 1], f32)
        nc.vector.memset(c1024, SH)
        c16 = const.tile([P, 1], f32)
        nc.vector.memset(c16, 16.0)
        cinv16 = const.tile([P, 1], f32)
        nc.vector.memset(cinv16, 1.0 / 16.0)
        qb1_s = const.tile([P, DSUB], f32)
        nc.sync.dma_start(qb1_s, qb1_dr.rearrange("(c p) -> p c", p=P))
        kb1_s = const.tile([P, DSUB], f32)
        nc.sync.dma_start(kb1_s, kb1_dr.rearrange("(c p) -> p c", p=P))
        qb2_s = const.tile([P, DSUB], f32)
        nc.sync.dma_start(qb2_s, qb2_dr.rearrange("(c p) -> p c", p=P))
        bffh_s = const.tile([P, FSUB], f32)
        nc.sync.dma_start(bffh_s, bffh_dr.rearrange("(c p) -> p c", p=P))
        bffg_s = const.tile([P, FSUB], f32)
        nc.sync.dma_start(bffg_s, bffg_dr.rearrange("(c p) -> p c", p=P))
        mcol_s = const.tile([P, TSUB], f32)
        nc.sync.dma_start(mcol_s, mcol_dr.rearrange("(c p) -> p c", p=P))
        bo1_b = bo2_b = bff2_b = vb1_b = None
        if with_vb1:
            vb1_b = const.tile([P, D], f32)
            nc.sync.dma_start(vb1_b, vb1_dr[None, :].to_broadcast((P, D)))
        if with_bo1:
            bo1_b = const.tile([P, D], f32)
            nc.sync.dma_start(bo1_b, bo1_dr[None, :].to_broadcast((P, D)))
        if with_bo2:
            bo2_b = const.tile([P, D], f32)
            nc.sync.dma_start(bo2_b, bo2_dr[None, :].to_broadcast((P, D)))
        if with_bff2:
            bff2_b = const.tile([P, D], f32)
            nc.sync.dma_start(bff2_b, bff2_dr[None, :].to_broadcast((P, D)))

        # ---------- persistent tiles (freed in LIFO order) ----------
        hbuf, free_hbuf = tc.tile([P, QSUB, D], f32, name="hbuf")
        xn3T8, free_xn3T8 = tc.tile([P, DSUB, SQ], fp8, name="xn3T8")
        xn3loT8, free_xn3loT8 = tc.tile([P, DSUB, SQ], fp8, name="xn3loT8")
        q1_8, free_q1 = tc.tile([P, 4, 2, SQ], fp8, name="q1_8")
        k1_8, free_k1 = tc.tile([P, 4, 2, T], fp8, name="k1_8")
        v1_8, free_v1 = tc.tile([P, TSUB, H, 65], fp8, name="v1_8")
        k2_8, free_k2 = tc.tile([P, 4, 2, T], fp8, name="k2_8")
        v2_8, free_v2 = tc.tile([P, TSUB, H, 65], fp8, name="v2_8")

        # ---------- preamble: LN1 over full S + transposes ----------
        w1_es = contextlib.ExitStack()
        wpool1 = w1_es.enter_context(tc.tile_pool(name="w1", bufs=3))
        wq1_sb = wpool1.tile([P, DSUB, D], fp8, tag="wq1")
        nc.sync.dma_start(wq1_sb, w_t["Wq1"])
        wk1_sb = wpool1.tile([P, DSUB, D], fp8, tag="wk1")
        nc.sync.dma_start(wk1_sb, w_t["Wk1"])
        wv1_sb = wpool1.tile([P, DSUB, D], fp8, tag="wv1")
        nc.sync.dma_start(wv1_sb, w_t["Wv1"])
        xn1T8, free_xn1T8 = tc.tile([P, DSUB, S], fp8, name="xn1T8")

        if _KSTOP >= 1:
         with (
            nc.named_scope("ln1"),
            tc.tile_pool(name="x_in", bufs=3) as x_pool,
            tc.tile_pool(name="xn1", bufs=3) as xn1_pool,
            tc.tile_pool(name="tr1_ps", bufs=2, space="PSUM") as tr1_ps,
            tc.tile_pool(name="qkv1_ps", bufs=4, space="PSUM") as ps_proj,
         ):
            def _q1(j):
                ps = ps_proj.tile([P, 512], f32, tag="proj")
                _proj_dr(nc, ps, wq1_sb, xn1T8, j, SQ)
                if with_qb1:
                    nc.scalar.activation(
                        q1_8[:, j // 2, j % 2, :], ps, AF.Identity,
                        bias=qb1_s[:, j:j + 1])
                else:
                    nc.scalar.copy(q1_8[:, j // 2, j % 2, :], ps)

            def _k1(j, tch):
                ps = ps_proj.tile([P, 512], f32, tag="proj")
                _proj_dr(nc, ps, wk1_sb, xn1T8, j, 512, col0=tch * 512)
                dst = k1_8[:, j // 2, j % 2, tch * 512:(tch + 1) * 512]
                if with_kb1:
                    nc.scalar.activation(
                        dst, ps, AF.Identity, bias=kb1_s[:, j:j + 1])
                else:
                    nc.scalar.copy(dst, ps)

            def _v1(ts, dh):
                ps = ps_proj.tile([P, 512], f32, tag="proj")
                for ksp in range(4):
                    nc.tensor.matmul(
                        ps,
                        xn1T8[:, 2 * ksp:2 * ksp + 2, ts * P:(ts + 1) * P],
                        wv1_sb[:, 2 * ksp:2 * ksp + 2,
                               dh * 512:(dh + 1) * 512],
                        start=(ksp == 0), stop=(ksp == 3), perf_mode=DR)
                dst = v1_8[:, ts, dh * 8:(dh + 1) * 8, 0:64]
                src = ps.rearrange("p (h w) -> p h w", h=8)
                if with_vb1:
                    nc.vector.tensor_tensor(
                        dst, src,
                        vb1_b[:, dh * 512:(dh + 1) * 512].rearrange(
                            "p (h w) -> p h w", h=8),
                        ALU.add)
                else:
                    nc.scalar.copy(dst, src)

            # qkv1 work queued behind the ln1 tiles it depends on: q1 and
            # the first k1/v1 halves need only rows 0..511 (ss 0..3).
            half1 = ([lambda j=j: _q1(j) for j in range(DSUB)]
                     + [lambda j=j: _k1(j, 0) for j in range(DSUB)]
                     + [lambda ts=ts, dh=dh: _v1(ts, dh)
                        for ts in range(QSUB) for dh in range(2)])
            half2 = ([lambda j=j: _k1(j, 1) for j in range(DSUB)]
                     + [lambda ts=ts, dh=dh: _v1(ts, dh)
                        for ts in range(QSUB, TSUB) for dh in range(2)])

            for ss in range(SSUB):
                xt = x_pool.tile([P, D], f32, tag="x")
                nc.sync.dma_start(xt, x_tiled[:, ss])
                mv, rstd = _ln_stats(nc, sb_small, xt, eps_ap)
                xn = xn1_pool.tile([P, D], bf16, tag="xn1")
                _ln_norm(nc, xt, xn, mv, rstd)
                if ss < QSUB:
                    nc.gpsimd.tensor_scalar(
                        hbuf[:, ss], xt, c1024, None, ALU.mult)
                    if with_bo1:
                        nc.gpsimd.tensor_tensor(
                            hbuf[:, ss], hbuf[:, ss], bo1_b, ALU.add)
                _transpose8(nc, tr1_ps, identb, xn,
                            xn1T8[:, :, ss * P:(ss + 1) * P])
                if _KSTOP >= 2:
                    if 4 <= ss < 7:
                        for _ in range(8):
                            half1.pop(0)()
                    elif ss == 7:
                        for fn in half1:
                            fn()
            if _KSTOP >= 2:
                for fn in half2:
                    fn()
                nc.gpsimd.memset(v1_8[:, :, :, 64:65], C64)
        free_xn1T8()
        w1_es.close()

        # weights/inputs needed during the main loop
        wo1_sb, free_wo1 = tc.tile([P, DSUB, D], fp8, name="wo1")
        nc.sync.dma_start(wo1_sb, w_t["Wo1"])
        wq2_sb, free_wq2 = tc.tile([P, DSUB, D], fp8, name="wq2")
        nc.sync.dma_start(wq2_sb, w_t["Wq2"])
        wo2_sb, free_wo2 = tc.tile([P, DSUB, D], fp8, name="wo2")
        nc.sync.dma_start(wo2_sb, w_t["Wo2"])
        ctx_es = contextlib.ExitStack()
        ctxpool = ctx_es.enter_context(tc.tile_pool(name="ctx", bufs=1))
        ctxT_sb = ctxpool.tile([P, DSUB, T], fp8, tag="ctxT")
        nc.sync.dma_start(ctxT_sb, ctxT_dr.rearrange("(ds p) t -> p ds t", p=P))
        ctxTm_sb = ctxpool.tile([P, DSUB, T], fp8, tag="ctxTm")
        nc.sync.dma_start(ctxTm_sb, ctxTm_dr.rearrange("(ds p) t -> p ds t", p=P))
        wk2_sb = ctxpool.tile([P, DSUB, D], fp8, tag="wk2")
        nc.sync.dma_start(wk2_sb, w_t["Wk2"])
        wv2_sb = ctxpool.tile([P, DSUB, D], fp8, tag="wv2")
        nc.sync.dma_start(wv2_sb, w_t["Wv2"])

        # ---------- main per-sc loop ----------
        main_es = contextlib.ExitStack()
        sc_pool = main_es.enter_context(
            tc.tile_pool(name="scores", bufs=2, space="PSUM"))
        pv_pool = main_es.enter_context(
            tc.tile_pool(name="pv", bufs=1, space="PSUM"))
        tr_ps = main_es.enter_context(
            tc.tile_pool(name="tr_ps", bufs=1, space="PSUM"))
        proj_ps = main_es.enter_context(
            tc.tile_pool(name="proj_ps", bufs=1, space="PSUM"))
        et_pool = main_es.enter_context(tc.tile_pool(name="ET", bufs=3))
        nat_pool = main_es.enter_context(tc.tile_pool(name="nat", bufs=2))
        aT_pool = main_es.enter_context(tc.tile_pool(name="aT", bufs=2))
        q2_pool = main_es.enter_context(tc.tile_pool(name="q2", bufs=2))

        # k2/v2 chunk emitters, interleaved into attn1 of sc 0/1
        k2v2_chunks = []

        def _k2_chunk(j, tch):
            def emit():
                ps = proj_ps.tile([P, 512], f32, tag="mps")
                _proj_dr(nc, ps, wk2_sb, ctxT_sb, j, 512, col0=tch * 512)
                nc.vector.tensor_copy(
                    k2_8[:, j // 2, j % 2, tch * 512:(tch + 1) * 512], ps)
            return emit

        def _v2_chunk(ts, dh):
            def emit():
                ps = proj_ps.tile([P, 512], f32, tag="mps")
                for ksp in range(4):
                    nc.tensor.matmul(
                        ps,
                        ctxTm_sb[:, 2 * ksp:2 * ksp + 2, ts * P:(ts + 1) * P],
                        wv2_sb[:, 2 * ksp:2 * ksp + 2, dh * 512:(dh + 1) * 512],
                        start=(ksp == 0), stop=(ksp == 3), perf_mode=DR)
                nc.vector.tensor_copy(
                    v2_8[:, ts, dh * 8:(dh + 1) * 8, 0:64],
                    ps.rearrange("p (h w) -> p h w", h=8))
                if dh == 1:
                    nc.gpsimd.tensor_copy(
                        v2_8[:, ts, :, 64],
                        mcol_s[:, ts:ts + 1].to_broadcast((P, H)))
            return emit

        for j in range(DSUB):
            for tch in range(2):
                k2v2_chunks.append(_k2_chunk(j, tch))
        for ts in range(TSUB):
            for dh in range(2):
                k2v2_chunks.append(_v2_chunk(ts, dh))
        k2v2_pos = [0]

        def interleave(h):
            # All 32 chunks must be emitted before attn2(sc0) is emitted:
            # 2 per head over attn1(sc0)'s 16 heads.
            budget = 2
            while budget and k2v2_pos[0] < len(k2v2_chunks):
                k2v2_chunks[k2v2_pos[0]]()
                k2v2_pos[0] += 1
                budget -= 1

        def no_interleave(h):
            pass

        attn_pools = (sc_pool, et_pool, pv_pool, sb_small)

        def q1_slices(p0, a, sc):
            return q1_8[p0:p0 + 32, a, :, sc * P:(sc + 1) * P]

        def _wo_block(sc, attn_nat, wo_sb, bias_b):
            aT = aT_pool.tile([P, DSUB, P], fp8, tag="aT")
            _transpose8(nc, tr_ps, identb, attn_nat, aT)
            for dh in range(2):
                ps = proj_ps.tile([P, 512], f32, tag="mps")
                for ksp in range(4):
                    nc.tensor.matmul(
                        ps,
                        aT[:, 2 * ksp:2 * ksp + 2, :],
                        wo_sb[:, 2 * ksp:2 * ksp + 2, dh * 512:(dh + 1) * 512],
                        start=(ksp == 0), stop=(ksp == 3), perf_mode=DR)
                sl = slice(dh * 512, (dh + 1) * 512)
                nc.vector.tensor_tensor(
                    hbuf[:, sc, sl], ps, hbuf[:, sc, sl], ALU.add)
            if bias_b is not None:
                nc.gpsimd.tensor_tensor(
                    hbuf[:, sc], hbuf[:, sc], bias_b, ALU.add)

        def _ln_block(sc, xnT8_dst, xlo_dst=None):
            mv, rstd = _ln_stats(nc, sb_small, hbuf[:, sc], eps_ap)
            xn = nat_pool.tile([P, D], bf16, tag="xn")
            _ln_norm(nc, hbuf[:, sc], xn, mv, rstd)
            for half in range(2):
                tp = tr_ps.tile([P, 4, P], bf16, tag="tr")
                for i in range(4):
                    dsb = half * 4 + i
                    nc.tensor.transpose(
                        tp[:, i], xn[:, dsb * P:(dsb + 1) * P], identb)
                dst = xnT8_dst[:, half * 4:(half + 1) * 4, :]
                nc.vector.tensor_copy(dst, tp)
                if xlo_dst is not None:
                    # fp8 residual of the quantization, scaled x16
                    t = nat_pool.tile([P, 4, P], bf16, tag="xlo_t")
                    nc.vector.tensor_tensor(t, tp, dst, ALU.subtract)
                    nc.vector.tensor_scalar(
                        xlo_dst[:, half * 4:(half + 1) * 4, :], t, c16, None,
                        ALU.mult)

        a1_nats = {}

        def _block2(sc):
            """wo1 -> ln2 -> q2 -> attn2 -> wo2 -> ln3 for one sc chunk."""
            if _KSTOP >= 4:
             with nc.named_scope("wo1"):
                _wo_block(sc, a1_nats.pop(sc), wo1_sb, bo1_b)
            if _KSTOP >= 5:
             with nc.named_scope("ln2"):
                xn2T8 = aT_pool.tile([P, DSUB, P], fp8, tag="xn2T")
                _ln_block(sc, xn2T8)
             with nc.named_scope("q2"):
                q2t = q2_pool.tile([P, 4, 2, P], fp8, tag="q2")
                for jp in range(2):
                    ps = proj_ps.tile([P, 512], f32, tag="mps")
                    psv = ps.rearrange("p (i s) -> p i s", i=4)
                    for i in range(4):
                        _proj_dr(nc, psv[:, i], wq2_sb, xn2T8, jp * 4 + i, P)
                    dst = q2t[:, 2 * jp:2 * jp + 2, :, :]
                    src = ps.rearrange("p (a k s) -> p a k s", a=2, k=2)
                    if with_qb2:
                        nc.vector.tensor_tensor(
                            dst, src,
                            qb2_s[:, 4 * jp:4 * jp + 4].rearrange(
                                "p (a k) -> p a k", a=2
                            )[:, :, :, None].to_broadcast((P, 2, 2, P)),
                            ALU.add)
                    else:
                        nc.vector.tensor_copy(dst, src)
            if _KSTOP >= 6:
             with nc.named_scope("attn2"):
                attn2_nat = nat_pool.tile([P, D], bf16, tag="a2nat")

                def q2_slices(p0, a, _sc, q2t=q2t):
                    return q2t[p0:p0 + 32, a, :, :]

                _attn_heads(nc, attn_pools, k2_8, v2_8, q2_slices, attn2_nat,
                            sc, no_interleave)
            if _KSTOP >= 7:
             with nc.named_scope("wo2"):
                _wo_block(sc, attn2_nat, wo2_sb, bo2_b)
            if _KSTOP >= 8:
             with nc.named_scope("ln3"):
                _ln_block(sc, xn3T8[:, :, sc * P:(sc + 1) * P],
                          xn3loT8[:, :, sc * P:(sc + 1) * P])

        # Skewed pipeline: block2(sc-1) is emitted after attn1(sc), so the
        # wo1->ln2->q2 chain of sc-1 resolves while ACT runs exp1(sc).
        for sc in range(QSUB):
            if _KSTOP >= 3:
             with nc.named_scope("attn1"):
                a1_nats[sc] = nat_pool.tile([P, D], bf16, tag="a1nat",
                                            name=f"a1nat{sc}")
                _attn_heads(nc, attn_pools, k1_8, v1_8, q1_slices,
                            a1_nats[sc], sc,
                            interleave if sc == 0 else no_interleave)
            if sc >= 1:
                _block2(sc - 1)
        if _KSTOP >= 3:
            _block2(QSUB - 1)

        main_es.close()
        ctx_es.close()
        free_wo2(); free_wq2(); free_wo1()
        free_v2(); free_k2(); free_v1(); free_k1(); free_q1()

        # ---------- GEGLU feed-forward ----------
        # FF1 PSUM carries 256*(xn@Wff1) via three accumulated fp8 terms:
        # xn8 @ (16*Whi) + xn8 @ Wlo + xlo8 @ Whi, with Wlo = fp8 residual
        # of 16*Wff1 scaled x16 and xlo8 the x16 fp8 residual of xn.
        mT8, free_mT8 = tc.tile([P, FSUB, SQ], fp8, name="mT8")
        wff2_sb, free_wff2 = tc.tile([P, FSUB, D], fp8, name="wff2")
        nc.sync.dma_start(wff2_sb, wff2_dr.rearrange("(ks p) o -> p ks o", p=P))
        wff1_t = wff1_dr.rearrange("(ks p) f -> p ks f", p=P)
        wff1l_t = wff1l_dr.rearrange("(ks p) f -> p ks f", p=P)
        wff1h_t = wff1h_dr.rearrange("(ks p) f -> p ks f", p=P)
        if _KSTOP >= 9:
         with (
            nc.named_scope("ff1"),
            tc.tile_pool(name="wff1", bufs=2) as wff1_pool,
            tc.tile_pool(name="ff1_ps", bufs=2, space="PSUM") as ps_ff1,
            tc.tile_pool(name="hT", bufs=3) as hT_pool,
            tc.tile_pool(name="gT", bufs=3) as gT_pool,
         ):
            def _load3(c0, sfx):
                w16 = wff1_pool.tile([P, DSUB, D], fp8, tag="w16" + sfx)
                nc.sync.dma_start(w16, wff1_t[:, :, c0:c0 + D])
                wlo = wff1_pool.tile([P, DSUB, D], fp8, tag="wlo" + sfx)
                nc.sync.dma_start(wlo, wff1l_t[:, :, c0:c0 + D])
                whi = wff1_pool.tile([P, DSUB, D], fp8, tag="whi" + sfx)
                nc.sync.dma_start(whi, wff1h_t[:, :, c0:c0 + D])
                return w16, wlo, whi

            def _ff1_mm(ps, w3, fi, rhs_hi, rhs_lo):
                w16, wlo, whi = w3
                _proj_dr(nc, ps, w16, rhs_hi, fi, SQ, last=False)
                _proj_dr(nc, ps, wlo, rhs_hi, fi, SQ, first=False, last=False)
                _proj_dr(nc, ps, whi, rhs_lo, fi, SQ, first=False)

            for cc in range(4):
                wh3 = _load3(cc * D, "h")
                wg3 = _load3(FF + cc * D, "g")
                for fi in range(DSUB):
                    fc = cc * DSUB + fi
                    ps_h = ps_ff1.tile([P, SQ], f32, tag="psh")
                    _ff1_mm(ps_h, wh3, fi, xn3T8, xn3loT8)
                    ps_g = ps_ff1.tile([P, SQ], f32, tag="psg")
                    _ff1_mm(ps_g, wg3, fi, xn3T8, xn3loT8)
                    hT = hT_pool.tile([P, SQ], bf16, tag="hT")
                    nc.vector.tensor_scalar(
                        hT, ps_h, cinv16, bffh_s[:, fc:fc + 1],
                        ALU.mult, ALU.add)
                    gT = gT_pool.tile([P, SQ], bf16, tag="gT")
                    nc.scalar.activation(
                        gT, ps_g, AF.Gelu, bias=bffg_s[:, fc:fc + 1],
                        scale=1.0 / (SW * 16.0))
                    eng = nc.gpsimd if fc % 2 else nc.vector
                    eng.tensor_tensor(mT8[:, fc, :], hT, gT, ALU.mult)

        if _KSTOP >= 10:
         with (
            nc.named_scope("ff2"),
            tc.tile_pool(name="ff2_ps", bufs=1, space="PSUM") as ps_ff2,
         ):
            ps_o = [ps_ff2.tile([P, 512], f32, tag=f"o{i}", name=f"ps_o{i}")
                    for i in range(8)]
            for ksp in range(FSUB // 2):
                for sc in range(QSUB):
                    for dh in range(2):
                        nc.tensor.matmul(
                            ps_o[sc * 2 + dh],
                            mT8[:, 2 * ksp:2 * ksp + 2, sc * P:(sc + 1) * P],
                            wff2_sb[:, 2 * ksp:2 * ksp + 2,
                                    dh * 512:(dh + 1) * 512],
                            start=(ksp == 0), stop=(ksp == FSUB // 2 - 1),
                            perf_mode=DR)
            for sc in range(QSUB):
                if with_bff2:
                    nc.gpsimd.tensor_tensor(
                        hbuf[:, sc], hbuf[:, sc], bff2_b, ALU.add)
                for dh in range(2):
                    sl = slice(dh * 512, (dh + 1) * 512)
                    nc.vector.tensor_tensor(
                        hbuf[:, sc, sl], ps_o[sc * 2 + dh],
                        hbuf[:, sc, sl], ALU.add)
        for sc in range(QSUB):
            nc.sync.dma_start(
                out_dr.rearrange("(ss p) d -> p ss d", p=P)[:, sc],
                hbuf[:, sc])
        free_wff2(); free_mT8()
        free_xn3loT8(); free_xn3T8(); free_hbuf()

    nc.compile()
    return nc


# --------------------------------------------------------------------------
# host side
# --------------------------------------------------------------------------

_NC = None
_NC_FLAGS = None


def _perm_qk():
    """Column permutation for Wq/Wk: head-interleaved DoubleRow layout."""
    perm = np.empty(D, np.int64)
    for h in range(H):
        for dl in range(DH):
            j = (h // 4) * 2 + (dl // 32)
            c = (h % 4) * 32 + (dl % 32)
            perm[j * P + c] = h * DH + dl
    return perm


_PERM = _perm_qk()


def _q8(x, scale=1.0):
    x = np.asarray(x, np.float32) * scale
    am = np.abs(x).max()
    assert am < 440, f"fp8 overflow: absmax {am}"
    return x.astype(FP8T)


def _get_nc(flags=()):
    global _NC, _NC_FLAGS
    flags = tuple(flags)
    if _NC is None or _NC_FLAGS != flags:
        _NC = build_nc(**dict(flags))
        _NC_FLAGS = flags
    return _NC


def _make_in_maps(inputs):
    f = np.float32
    hidden = np.asarray(inputs["hidden_states"], f)
    context = np.asarray(inputs["context"], f)
    mask = np.asarray(inputs["encoder_key_padding_mask"]).astype(f)
    g1, b1 = np.asarray(inputs["g1"], f), np.asarray(inputs["b1"], f)
    g2, b2 = np.asarray(inputs["g2"], f), np.asarray(inputs["b2"], f)
    g3, b3 = np.asarray(inputs["g3"], f), np.asarray(inputs["b3"], f)

    def fold(g, W):
        return g[:, None] * np.asarray(W, f)

    Wq1 = fold(g1, inputs["Wq1"])[:, _PERM]
    Wk1 = fold(g1, inputs["Wk1"])[:, _PERM]
    Wv1 = fold(g1, inputs["Wv1"])
    Wo1 = np.asarray(inputs["Wo1"], f)
    qb1 = (b1 @ np.asarray(inputs["Wq1"], f))[_PERM]
    kb1 = (b1 @ np.asarray(inputs["Wk1"], f))[_PERM]
    vb1 = b1 @ np.asarray(inputs["Wv1"], f)
    Wq2 = fold(g2, inputs["Wq2"])[:, _PERM]
    Wk2 = np.asarray(inputs["Wk2"], f)[:, _PERM]
    Wv2 = np.asarray(inputs["Wv2"], f)
    Wo2 = np.asarray(inputs["Wo2"], f)
    qb2 = (b2 @ np.asarray(inputs["Wq2"], f))[_PERM]
    Wff1 = fold(g3, inputs["Wff1"])
    bff1 = np.asarray(inputs["bff1"], f) + b3 @ np.asarray(inputs["Wff1"], f)
    Wff2 = np.asarray(inputs["Wff2"], f)
    bo1 = np.asarray(inputs["bo1"], f)
    bo2 = np.asarray(inputs["bo2"], f)
    bff2 = np.asarray(inputs["bff2"], f)

    flags = (
        ("with_vb1", bool(np.any(vb1))),
        ("with_bo1", bool(np.any(bo1))),
        ("with_bo2", bool(np.any(bo2))),
        ("with_bff2", bool(np.any(bff2))),
        ("with_qb1", bool(np.any(qb1))),
        ("with_kb1", bool(np.any(kb1))),
        ("with_qb2", bool(np.any(qb2))),
    )

    wff1_hi = _q8(Wff1, SW)
    wff1_hi_f = wff1_hi.astype(f)
    wff1_hi16 = _q8(wff1_hi_f, 16.0)          # exact exponent shift
    wff1_lo = _q8((SW * Wff1 - wff1_hi_f), 16.0)

    shared = {
        "Wq1": _q8(Wq1, SW), "Wk1": _q8(Wk1, SW), "Wv1": _q8(Wv1, SW),
        "Wo1": _q8(Wo1, SW),
        "Wq2": _q8(Wq2, SW), "Wk2": _q8(Wk2, SW), "Wv2": _q8(Wv2, SW),
        "Wo2": _q8(Wo2, SW),
        "Wff1": wff1_hi16, "Wff1l": wff1_lo, "Wff1h": wff1_hi,
        "Wff2": _q8(Wff2, 64.0),
        "qb1_16": np.ascontiguousarray(SW * qb1),
        "kb1_16": np.ascontiguousarray(SW * kb1),
        "qb2_16": np.ascontiguousarray(SW * qb2),
        "vb1_16": np.ascontiguousarray(SW * vb1),
        "bo1_s": np.ascontiguousarray(SH * bo1),
        "bo2_s": np.ascontiguousarray(SH * bo2),
        "bff2_s": np.ascontiguousarray(SH * bff2),
        "bffh_16": np.ascontiguousarray(SW * bff1[:FF]),
        "bffg": np.ascontiguousarray(bff1[FF:]),
    }

    in_maps = []
    for core in range(NCORES):
        b, q = core // 2, core % 2
        x = hidden[b] if q == 0 else np.roll(hidden[b], -SQ, axis=0)
        ctxT = np.ascontiguousarray(context[b].T)
        ctxTm = np.ascontiguousarray((mask[b][:, None] * context[b]).T)
        in_maps.append({
            **shared,
            "x": np.ascontiguousarray(x),
            "ctxT8": _q8(ctxT),
            "ctxTm8": _q8(ctxTm),
            "mcol": np.ascontiguousarray(C64 * mask[b]),
        })
    return in_maps, flags


def run(inputs, **spmd_kwargs):
    in_maps, flags = _make_in_maps(inputs)
    res = run_bass_kernel_spmd(
        _get_nc(flags), in_maps, core_ids=list(range(NCORES)),
        **spmd_kwargs)
    out = np.empty((B, S, D), np.float32)
    for core in range(NCORES):
        b, q = core // 2, core % 2
        out[b, q * SQ:(q + 1) * SQ] = res.results[core]["out"] * (1.0 / SH)
    return out, res


def kernel(**inputs):
    out, _ = run(inputs)
    return out


# revision 33
# speedup vs baseline: 1.4439x; 1.0007x over previous
"""BasicTransformerBlock (self-attn + cross-attn + GEGLU FF) on 8 TRN2 cores.

Sharding: sequence-parallel, no collectives. B=4 batches x 2 sequence-halves
= 8 shards; each core computes 512 query rows end-to-end, duplicating only
the K/V projections for its batch. The host rolls each batch's hidden_states
so a core's query rows are always rows 0..511 - the kernel is uniform SPMD.

v2: all matmuls in fp8 e4m3 with DoubleRow perf mode (0.5 cyc/row, 256-deep
contraction per instruction). Fixed power-of-2 scale conventions:
  - weights quantized x16 host-side (x64 for Wff2)
  - activations (xn) quantized at true scale; q8/k8 carry 16x and the
    exp scale absorbs 1/256
  - V columns carry 16x, fused denominator column = 0.25 so the normalize
    step yields 64x attention output (good fp8 range)
  - attn(64x) @ Wo(16x) -> PSUM carries 1024x; the residual buffer hbuf
    holds 1024*h throughout (LayerNorm is scale-invariant); the host
    divides the final output by 1024 (exact)
  - FF: hT=16h', gT=gelu(g) true, mT=16m, Wff2 x64 -> PSUM 1024x
Scores for head h contract over d=64 as [32 partitions x 2 DoubleRow
k-tiles]; Wq/Wk columns are permuted host-side so the projections emit q/k
directly in that layout. Transposes run on PE in bf16 (fp8 transpose is
rejected by the walrus verifier); the PSUM->SBUF copy converts to fp8.

Engines: ACT = exp + gelu + q1/k1 evac; DVE = LN, PSUM evacs, PV normalize;
Pool (gpsimd, SBUF-only) = half the ff multiplies, hbuf scaling, fills.
Program order pipelines per 128-row query chunk (sc) so ACT stays saturated
with exp through both attention blocks.
"""

import contextlib
import os

import numpy as np
import ml_dtypes

_KSTOP = int(os.environ.get("KSTOP", "99"))

import concourse.mybir as mybir
import concourse.tile as tile
from concourse import bacc
from concourse.bass_utils import run_bass_kernel_spmd
from concourse.masks import make_identity

P = 128
B, S, T, D, H, DH = 4, 1024, 1024, 1024, 16, 64
FF = 4 * D
SQ = 512                 # query rows per core
SCALE = DH ** -0.5
EPS = 1e-12
NCORES = 8

f32 = mybir.dt.float32
bf16 = mybir.dt.bfloat16
fp8 = mybir.dt.float8e4
AF = mybir.ActivationFunctionType
ALU = mybir.AluOpType
DR = mybir.MatmulPerfMode.DoubleRow

DSUB = D // P            # 8
TSUB = T // P            # 8
SSUB = S // P            # 8
QSUB = SQ // P           # 4
FSUB = FF // P           # 32

SW = 16.0                # weight quant scale (x64 for Wff2)
SEXP = SCALE / 256.0     # exp scale: q,k both carry 16x
C64 = 0.25               # fused denominator column value -> attn_nat = 64x
SH = 1024.0              # hbuf carries 1024*h

FP8T = ml_dtypes.float8_e4m3fn


# --------------------------------------------------------------------------
# device-program helpers
# --------------------------------------------------------------------------

def _ln_stats(nc, sb_small, x_ap, eps_ap):
    """Return (mv, rstd): per-row mean/var and 1/sqrt(var+eps)."""
    stats = sb_small.tile([P, D // 512, 6], f32, tag="ln_stats")
    for c in range(D // 512):
        nc.vector.bn_stats(stats[:, c], x_ap[:, c * 512:(c + 1) * 512])
    mv = sb_small.tile([P, 2], f32, tag="ln_mv")
    nc.vector.bn_aggr(mv, stats)
    std = sb_small.tile([P, 1], f32, tag="ln_std")
    nc.scalar.activation(std, mv[:, 1:2], AF.Sqrt, bias=eps_ap)
    rstd = sb_small.tile([P, 1], f32, tag="ln_rstd")
    nc.vector.reciprocal(rstd, std)
    return mv, rstd


def _ln_norm(nc, x_ap, xn_ap, mv, rstd):
    for c in range(2):
        sl = slice(c * (D // 2), (c + 1) * (D // 2))
        nc.vector.tensor_scalar(xn_ap[:, sl], x_ap[:, sl], mv[:, 0:1], rstd,
                                ALU.subtract, ALU.mult)


def _transpose8(nc, tr_ps, identb, src_nat, dst8):
    """PE-transpose src_nat [P, D] bf16 into dst8 [P, 8, 128] fp8."""
    for half in range(2):
        tp = tr_ps.tile([P, 4, P], bf16, tag="tr")
        for i in range(4):
            dsb = half * 4 + i
            nc.tensor.transpose(
                tp[:, i], src_nat[:, dsb * P:(dsb + 1) * P], identb)
        nc.vector.tensor_copy(dst8[:, half * 4:(half + 1) * 4, :], tp)


def _proj_dr(nc, ps, w8, rhs8, j, ncols, col0=0, first=True, last=True):
    """ps[128, ncols] (+)= (W block j).T @ rhs over D=1024 (4 DoubleRow mms)."""
    for ksp in range(4):
        nc.tensor.matmul(
            ps,
            w8[:, 2 * ksp:2 * ksp + 2, j * P:(j + 1) * P],
            rhs8[:, 2 * ksp:2 * ksp + 2, col0:col0 + ncols],
            start=(first and ksp == 0), stop=(last and ksp == 3),
            perf_mode=DR)


def _attn_heads(nc, pools, k8, v8, q8_slices, attn_nat, sc, interleave):
    """16 heads: scoresT -> exp -> PV(fused denom) -> normalize (per 8)."""
    sc_pool, et_pool, pv_pool, sb_small = pools
    pv = None
    for h in range(H):
        a, r = h // 4, h % 4
        p0 = r * 32
        ps_sc = sc_pool.tile([P, TSUB, P], f32, tag="sc")
        for tb in range(TSUB):
            nc.tensor.matmul(
                ps_sc[:, tb],
                k8[p0:p0 + 32, a, :, tb * P:(tb + 1) * P],
                q8_slices(p0, a, sc),
                start=True, stop=True, perf_mode=DR,
                tile_position=(p0, 0))
        ET = et_pool.tile([P, TSUB, P], fp8, tag="ET")
        nc.scalar.activation(ET, ps_sc, AF.Exp, scale=SEXP)
        if h % 8 == 0:
            pv = pv_pool.tile([P, 8, P], f32, tag="pv")
        for tsp in range(4):
            nc.tensor.matmul(
                pv[:, h % 8, 0:65],
                ET[:, 2 * tsp:2 * tsp + 2, :],
                v8[:, 2 * tsp:2 * tsp + 2, h, :],
                start=(tsp == 0), stop=(tsp == 3), perf_mode=DR)
        if h % 8 == 7:
            hh = h // 8
            rec = sb_small.tile([P, 8], f32, tag="pv_rec")
            nc.vector.reciprocal(rec, pv[:, :, 64])
            nc.vector.tensor_tensor(
                attn_nat[:, hh * 512:(hh + 1) * 512],
                pv[:, :, 0:64],
                rec[:, :, None].to_broadcast((P, 8, 64)),
                ALU.mult)
        interleave(h)


# --------------------------------------------------------------------------
# full program
# --------------------------------------------------------------------------

def build_nc(with_vb1=False, with_bo1=False, with_bo2=False, with_bff2=False,
             with_qb1=False, with_kb1=False, with_qb2=False):
    nc = bacc.Bacc(None, target_bir_lowering=False, debug=False)

    x_dr = nc.dram_tensor("x", [S, D], f32, kind="ExternalInput")
    ctxT_dr = nc.dram_tensor("ctxT8", [D, T], fp8, kind="ExternalInput")
    ctxTm_dr = nc.dram_tensor("ctxTm8", [D, T], fp8, kind="ExternalInput")
    mcol_dr = nc.dram_tensor("mcol", [T], f32, kind="ExternalInput")
    wdr = {}
    for nm in ("Wq1", "Wk1", "Wv1", "Wo1", "Wq2", "Wk2", "Wv2", "Wo2"):
        wdr[nm] = nc.dram_tensor(nm, [D, D], fp8, kind="ExternalInput")
    wff1_dr = nc.dram_tensor("Wff1", [D, 2 * FF], fp8, kind="ExternalInput")
    wff1l_dr = nc.dram_tensor("Wff1l", [D, 2 * FF], fp8, kind="ExternalInput")
    wff1h_dr = nc.dram_tensor("Wff1h", [D, 2 * FF], fp8, kind="ExternalInput")
    wff2_dr = nc.dram_tensor("Wff2", [FF, D], fp8, kind="ExternalInput")
    qb1_dr = nc.dram_tensor("qb1_16", [D], f32, kind="ExternalInput")
    kb1_dr = nc.dram_tensor("kb1_16", [D], f32, kind="ExternalInput")
    qb2_dr = nc.dram_tensor("qb2_16", [D], f32, kind="ExternalInput")
    vb1_dr = nc.dram_tensor("vb1_16", [D], f32, kind="ExternalInput")
    bo1_dr = nc.dram_tensor("bo1_s", [D], f32, kind="ExternalInput")
    bo2_dr = nc.dram_tensor("bo2_s", [D], f32, kind="ExternalInput")
    bff2_dr = nc.dram_tensor("bff2_s", [D], f32, kind="ExternalInput")
    bffh_dr = nc.dram_tensor("bffh_16", [FF], f32, kind="ExternalInput")
    bffg_dr = nc.dram_tensor("bffg", [FF], f32, kind="ExternalInput")
    out_dr = nc.dram_tensor("out", [SQ, D], f32, kind="ExternalOutput")

    x_tiled = x_dr.rearrange("(ss p) d -> p ss d", p=P)
    w_t = {k: v.rearrange("(ks p) o -> p ks o", p=P) for k, v in wdr.items()}

    with tile.TileContext(nc) as tc, contextlib.ExitStack() as es:
        const = es.enter_context(tc.tile_pool(name="const", bufs=1))
        sb_small = es.enter_context(tc.tile_pool(name="smalls", bufs=8))

        identb = const.tile([P, P], bf16)
        make_identity(nc, identb)
        eps_ap = const.tile([P, 1], f32)
        nc.vector.memset(eps_ap, EPS)
        c1024 = const.tile([P, 1], f32)
        nc.vector.memset(c1024, SH)
        c16 = const.tile([P, 1], f32)
        nc.vector.memset(c16, 16.0)
        cinv16 = const.tile([P, 1], f32)
        nc.vector.memset(cinv16, 1.0 / 16.0)
        qb1_s = const.tile([P, DSUB], f32)
        nc.sync.dma_start(qb1_s, qb1_dr.rearrange("(c p) -> p c", p=P))
        kb1_s = const.tile([P, DSUB], f32)
        nc.sync.dma_start(kb1_s, kb1_dr.rearrange("(c p) -> p c", p=P))
        qb2_s = const.tile([P, DSUB], f32)
        nc.sync.dma_start(qb2_s, qb2_dr.rearrange("(c p) -> p c", p=P))
        bffh_s = const.tile([P, FSUB], f32)
        nc.sync.dma_start(bffh_s, bffh_dr.rearrange("(c p) -> p c", p=P))
        bffg_s = const.tile([P, FSUB], f32)
        nc.sync.dma_start(bffg_s, bffg_dr.rearrange("(c p) -> p c", p=P))
        mcol_s = const.tile([P, TSUB], f32)
        nc.sync.dma_start(mcol_s, mcol_dr.rearrange("(c p) -> p c", p=P))
        bo1_b = bo2_b = bff2_b = vb1_b = None
        if with_vb1:
            vb1_b = const.tile([P, D], f32)
            nc.sync.dma_start(vb1_b, vb1_dr[None, :].to_broadcast((P, D)))
        if with_bo1:
            bo1_b = const.tile([P, D], f32)
            nc.sync.dma_start(bo1_b, bo1_dr[None, :].to_broadcast((P, D)))
        if with_bo2:
            bo2_b = const.tile([P, D], f32)
            nc.sync.dma_start(bo2_b, bo2_dr[None, :].to_broadcast((P, D)))
        if with_bff2:
            bff2_b = const.tile([P, D], f32)
            nc.sync.dma_start(bff2_b, bff2_dr[None, :].to_broadcast((P, D)))

        # ---------- persistent tiles (freed in LIFO order) ----------
        hbuf, free_hbuf = tc.tile([P, QSUB, D], f32, name="hbuf")
        xn3T8, free_xn3T8 = tc.tile([P, DSUB, SQ], fp8, name="xn3T8")
        xn3loT8, free_xn3loT8 = tc.tile([P, DSUB, SQ], fp8, name="xn3loT8")
        q1_8, free_q1 = tc.tile([P, 4, 2, SQ], fp8, name="q1_8")
        k1_8, free_k1 = tc.tile([P, 4, 2, T], fp8, name="k1_8")
        v1_8, free_v1 = tc.tile([P, TSUB, H, 65], fp8, name="v1_8")
        k2_8, free_k2 = tc.tile([P, 4, 2, T], fp8, name="k2_8")
        v2_8, free_v2 = tc.tile([P, TSUB, H, 65], fp8, name="v2_8")

        # ---------- preamble: LN1 over full S + transposes ----------
        w1_es = contextlib.ExitStack()
        wpool1 = w1_es.enter_context(tc.tile_pool(name="w1", bufs=3))
        wq1_sb = wpool1.tile([P, DSUB, D], fp8, tag="wq1")
        nc.sync.dma_start(wq1_sb, w_t["Wq1"])
        wk1_sb = wpool1.tile([P, DSUB, D], fp8, tag="wk1")
        nc.sync.dma_start(wk1_sb, w_t["Wk1"])
        wv1_sb = wpool1.tile([P, DSUB, D], fp8, tag="wv1")
        nc.sync.dma_start(wv1_sb, w_t["Wv1"])
        xn1T8, free_xn1T8 = tc.tile([P, DSUB, S], fp8, name="xn1T8")

        if _KSTOP >= 1:
         with (
            nc.named_scope("ln1"),
            tc.tile_pool(name="x_in", bufs=3) as x_pool,
            tc.tile_pool(name="xn1", bufs=3) as xn1_pool,
            tc.tile_pool(name="tr1_ps", bufs=4, space="PSUM") as tr1_ps,
            tc.tile_pool(name="qkv1_ps", bufs=4, space="PSUM") as ps_proj,
         ):
            def _q1(j):
                ps = ps_proj.tile([P, 512], f32, tag="proj")
                _proj_dr(nc, ps, wq1_sb, xn1T8, j, SQ)
                if with_qb1:
                    nc.scalar.activation(
                        q1_8[:, j // 2, j % 2, :], ps, AF.Identity,
                        bias=qb1_s[:, j:j + 1])
                else:
                    nc.scalar.copy(q1_8[:, j // 2, j % 2, :], ps)

            def _k1(j, tch):
                ps = ps_proj.tile([P, 512], f32, tag="proj")
                _proj_dr(nc, ps, wk1_sb, xn1T8, j, 512, col0=tch * 512)
                dst = k1_8[:, j // 2, j % 2, tch * 512:(tch + 1) * 512]
                if with_kb1:
                    nc.scalar.activation(
                        dst, ps, AF.Identity, bias=kb1_s[:, j:j + 1])
                else:
                    nc.scalar.copy(dst, ps)

            def _v1(ts, dh):
                ps = ps_proj.tile([P, 512], f32, tag="proj")
                for ksp in range(4):
                    nc.tensor.matmul(
                        ps,
                        xn1T8[:, 2 * ksp:2 * ksp + 2, ts * P:(ts + 1) * P],
                        wv1_sb[:, 2 * ksp:2 * ksp + 2,
                               dh * 512:(dh + 1) * 512],
                        start=(ksp == 0), stop=(ksp == 3), perf_mode=DR)
                dst = v1_8[:, ts, dh * 8:(dh + 1) * 8, 0:64]
                src = ps.rearrange("p (h w) -> p h w", h=8)
                if with_vb1:
                    nc.vector.tensor_tensor(
                        dst, src,
                        vb1_b[:, dh * 512:(dh + 1) * 512].rearrange(
                            "p (h w) -> p h w", h=8),
                        ALU.add)
                else:
                    nc.scalar.copy(dst, src)

            # qkv1 work queued behind the ln1 tiles it depends on: q1 and
            # the first k1/v1 halves need only rows 0..511 (ss 0..3).
            half1 = ([lambda j=j: _q1(j) for j in range(DSUB)]
                     + [lambda j=j: _k1(j, 0) for j in range(DSUB)]
                     + [lambda ts=ts, dh=dh: _v1(ts, dh)
                        for ts in range(QSUB) for dh in range(2)])
            half2 = ([lambda j=j: _k1(j, 1) for j in range(DSUB)]
                     + [lambda ts=ts, dh=dh: _v1(ts, dh)
                        for ts in range(QSUB, TSUB) for dh in range(2)])

            for ss in range(SSUB):
                xt = x_pool.tile([P, D], f32, tag="x")
                nc.sync.dma_start(xt, x_tiled[:, ss])
                mv, rstd = _ln_stats(nc, sb_small, xt, eps_ap)
                xn = xn1_pool.tile([P, D], bf16, tag="xn1")
                _ln_norm(nc, xt, xn, mv, rstd)
                if ss < QSUB:
                    nc.gpsimd.tensor_scalar(
                        hbuf[:, ss], xt, c1024, None, ALU.mult)
                    if with_bo1:
                        nc.gpsimd.tensor_tensor(
                            hbuf[:, ss], hbuf[:, ss], bo1_b, ALU.add)
                _transpose8(nc, tr1_ps, identb, xn,
                            xn1T8[:, :, ss * P:(ss + 1) * P])
                if _KSTOP >= 2:
                    if 4 <= ss < 7:
                        for _ in range(8):
                            half1.pop(0)()
                    elif ss == 7:
                        for fn in half1:
                            fn()
            if _KSTOP >= 2:
                for fn in half2:
                    fn()
                nc.gpsimd.memset(v1_8[:, :, :, 64:65], C64)
        free_xn1T8()
        w1_es.close()

        # weights/inputs needed during the main loop
        wo1_sb, free_wo1 = tc.tile([P, DSUB, D], fp8, name="wo1")
        nc.sync.dma_start(wo1_sb, w_t["Wo1"])
        wq2_sb, free_wq2 = tc.tile([P, DSUB, D], fp8, name="wq2")
        nc.sync.dma_start(wq2_sb, w_t["Wq2"])
        wo2_sb, free_wo2 = tc.tile([P, DSUB, D], fp8, name="wo2")
        nc.sync.dma_start(wo2_sb, w_t["Wo2"])
        ctx_es = contextlib.ExitStack()
        ctxpool = ctx_es.enter_context(tc.tile_pool(name="ctx", bufs=1))
        ctxT_sb = ctxpool.tile([P, DSUB, T], fp8, tag="ctxT")
        nc.sync.dma_start(ctxT_sb, ctxT_dr.rearrange("(ds p) t -> p ds t", p=P))
        ctxTm_sb = ctxpool.tile([P, DSUB, T], fp8, tag="ctxTm")
        nc.sync.dma_start(ctxTm_sb, ctxTm_dr.rearrange("(ds p) t -> p ds t", p=P))
        wk2_sb = ctxpool.tile([P, DSUB, D], fp8, tag="wk2")
        nc.sync.dma_start(wk2_sb, w_t["Wk2"])
        wv2_sb = ctxpool.tile([P, DSUB, D], fp8, tag="wv2")
        nc.sync.dma_start(wv2_sb, w_t["Wv2"])

        # ---------- main per-sc loop ----------
        main_es = contextlib.ExitStack()
        sc_pool = main_es.enter_context(
            tc.tile_pool(name="scores", bufs=2, space="PSUM"))
        pv_pool = main_es.enter_context(
            tc.tile_pool(name="pv", bufs=1, space="PSUM"))
        tr_ps = main_es.enter_context(
            tc.tile_pool(name="tr_ps", bufs=1, space="PSUM"))
        proj_ps = main_es.enter_context(
            tc.tile_pool(name="proj_ps", bufs=1, space="PSUM"))
        et_pool = main_es.enter_context(tc.tile_pool(name="ET", bufs=3))
        nat_pool = main_es.enter_context(tc.tile_pool(name="nat", bufs=2))
        aT_pool = main_es.enter_context(tc.tile_pool(name="aT", bufs=2))
        q2_pool = main_es.enter_context(tc.tile_pool(name="q2", bufs=2))

        # k2/v2 chunk emitters, interleaved into attn1 of sc 0/1
        k2v2_chunks = []

        def _k2_chunk(j, tch):
            def emit():
                ps = proj_ps.tile([P, 512], f32, tag="mps")
                _proj_dr(nc, ps, wk2_sb, ctxT_sb, j, 512, col0=tch * 512)
                nc.vector.tensor_copy(
                    k2_8[:, j // 2, j % 2, tch * 512:(tch + 1) * 512], ps)
            return emit

        def _v2_chunk(ts, dh):
            def emit():
                ps = proj_ps.tile([P, 512], f32, tag="mps")
                for ksp in range(4):
                    nc.tensor.matmul(
                        ps,
                        ctxTm_sb[:, 2 * ksp:2 * ksp + 2, ts * P:(ts + 1) * P],
                        wv2_sb[:, 2 * ksp:2 * ksp + 2, dh * 512:(dh + 1) * 512],
                        start=(ksp == 0), stop=(ksp == 3), perf_mode=DR)
                nc.vector.tensor_copy(
                    v2_8[:, ts, dh * 8:(dh + 1) * 8, 0:64],
                    ps.rearrange("p (h w) -> p h w", h=8))
                if dh == 1:
                    nc.gpsimd.tensor_copy(
                        v2_8[:, ts, :, 64],
                        mcol_s[:, ts:ts + 1].to_broadcast((P, H)))
            return emit

        for j in range(DSUB):
            for tch in range(2):
                k2v2_chunks.append(_k2_chunk(j, tch))
        for ts in range(TSUB):
            for dh in range(2):
                k2v2_chunks.append(_v2_chunk(ts, dh))
        k2v2_pos = [0]

        def interleave(h):
            # All 32 chunks must be emitted before _block2(0) (attn2 of sc0),
            # which the skewed loop places after attn1(sc1): 1 per head over
            # attn1(sc0..1)'s 32 head slots.
            budget = 1
            while budget and k2v2_pos[0] < len(k2v2_chunks):
                k2v2_chunks[k2v2_pos[0]]()
                k2v2_pos[0] += 1
                budget -= 1

        def no_interleave(h):
            pass

        attn_pools = (sc_pool, et_pool, pv_pool, sb_small)

        def q1_slices(p0, a, sc):
            return q1_8[p0:p0 + 32, a, :, sc * P:(sc + 1) * P]

        def _wo_block(sc, attn_nat, wo_sb, bias_b):
            aT = aT_pool.tile([P, DSUB, P], fp8, tag="aT")
            _transpose8(nc, tr_ps, identb, attn_nat, aT)
            for dh in range(2):
                ps = proj_ps.tile([P, 512], f32, tag="mps")
                for ksp in range(4):
                    nc.tensor.matmul(
                        ps,
                        aT[:, 2 * ksp:2 * ksp + 2, :],
                        wo_sb[:, 2 * ksp:2 * ksp + 2, dh * 512:(dh + 1) * 512],
                        start=(ksp == 0), stop=(ksp == 3), perf_mode=DR)
                sl = slice(dh * 512, (dh + 1) * 512)
                nc.vector.tensor_tensor(
                    hbuf[:, sc, sl], ps, hbuf[:, sc, sl], ALU.add)
            if bias_b is not None:
                nc.gpsimd.tensor_tensor(
                    hbuf[:, sc], hbuf[:, sc], bias_b, ALU.add)

        def _ln_block(sc, xnT8_dst, xlo_dst=None):
            mv, rstd = _ln_stats(nc, sb_small, hbuf[:, sc], eps_ap)
            xn = nat_pool.tile([P, D], bf16, tag="xn")
            _ln_norm(nc, hbuf[:, sc], xn, mv, rstd)
            for half in range(2):
                tp = tr_ps.tile([P, 4, P], bf16, tag="tr")
                for i in range(4):
                    dsb = half * 4 + i
                    nc.tensor.transpose(
                        tp[:, i], xn[:, dsb * P:(dsb + 1) * P], identb)
                dst = xnT8_dst[:, half * 4:(half + 1) * 4, :]
                nc.vector.tensor_copy(dst, tp)
                if xlo_dst is not None:
                    # fp8 residual of the quantization, scaled x16
                    t = nat_pool.tile([P, 4, P], bf16, tag="xlo_t")
                    nc.vector.tensor_tensor(t, tp, dst, ALU.subtract)
                    nc.vector.tensor_scalar(
                        xlo_dst[:, half * 4:(half + 1) * 4, :], t, c16, None,
                        ALU.mult)

        a1_nats = {}

        def _block2(sc):
            """wo1 -> ln2 -> q2 -> attn2 -> wo2 -> ln3 for one sc chunk."""
            if _KSTOP >= 4:
             with nc.named_scope("wo1"):
                _wo_block(sc, a1_nats.pop(sc), wo1_sb, bo1_b)
            if _KSTOP >= 5:
             with nc.named_scope("ln2"):
                xn2T8 = aT_pool.tile([P, DSUB, P], fp8, tag="xn2T")
                _ln_block(sc, xn2T8)
             with nc.named_scope("q2"):
                q2t = q2_pool.tile([P, 4, 2, P], fp8, tag="q2")
                for jp in range(2):
                    ps = proj_ps.tile([P, 512], f32, tag="mps")
                    psv = ps.rearrange("p (i s) -> p i s", i=4)
                    for i in range(4):
                        _proj_dr(nc, psv[:, i], wq2_sb, xn2T8, jp * 4 + i, P)
                    dst = q2t[:, 2 * jp:2 * jp + 2, :, :]
                    src = ps.rearrange("p (a k s) -> p a k s", a=2, k=2)
                    if with_qb2:
                        nc.vector.tensor_tensor(
                            dst, src,
                            qb2_s[:, 4 * jp:4 * jp + 4].rearrange(
                                "p (a k) -> p a k", a=2
                            )[:, :, :, None].to_broadcast((P, 2, 2, P)),
                            ALU.add)
                    else:
                        nc.vector.tensor_copy(dst, src)
            if _KSTOP >= 6:
             with nc.named_scope("attn2"):
                attn2_nat = nat_pool.tile([P, D], bf16, tag="a2nat")

                def q2_slices(p0, a, _sc, q2t=q2t):
                    return q2t[p0:p0 + 32, a, :, :]

                _attn_heads(nc, attn_pools, k2_8, v2_8, q2_slices, attn2_nat,
                            sc, no_interleave)
            if _KSTOP >= 7:
             with nc.named_scope("wo2"):
                _wo_block(sc, attn2_nat, wo2_sb, bo2_b)
            if _KSTOP >= 8:
             with nc.named_scope("ln3"):
                _ln_block(sc, xn3T8[:, :, sc * P:(sc + 1) * P],
                          xn3loT8[:, :, sc * P:(sc + 1) * P])

        # Skewed pipeline: block2(sc-1) is emitted after attn1(sc), so the
        # wo1->ln2->q2 chain of sc-1 resolves while ACT runs exp1(sc).
        for sc in range(QSUB):
            if _KSTOP >= 3:
             with nc.named_scope("attn1"):
                a1_nats[sc] = nat_pool.tile([P, D], bf16, tag="a1nat",
                                            name=f"a1nat{sc}")
                _attn_heads(nc, attn_pools, k1_8, v1_8, q1_slices,
                            a1_nats[sc], sc,
                            interleave if sc < 2 else no_interleave)
            if sc >= 1:
                _block2(sc - 1)
        if _KSTOP >= 3:
            _block2(QSUB - 1)

        main_es.close()
        ctx_es.close()
        free_wo2(); free_wq2(); free_wo1()
        free_v2(); free_k2(); free_v1(); free_k1(); free_q1()

        # ---------- GEGLU feed-forward ----------
        # FF1 PSUM carries 256*(xn@Wff1) via three accumulated fp8 terms:
        # xn8 @ (16*Whi) + xn8 @ Wlo + xlo8 @ Whi, with Wlo = fp8 residual
        # of 16*Wff1 scaled x16 and xlo8 the x16 fp8 residual of xn.
        mT8, free_mT8 = tc.tile([P, FSUB, SQ], fp8, name="mT8")
        wff2_sb, free_wff2 = tc.tile([P, FSUB, D], fp8, name="wff2")
        nc.sync.dma_start(wff2_sb, wff2_dr.rearrange("(ks p) o -> p ks o", p=P))
        wff1_t = wff1_dr.rearrange("(ks p) f -> p ks f", p=P)
        wff1l_t = wff1l_dr.rearrange("(ks p) f -> p ks f", p=P)
        wff1h_t = wff1h_dr.rearrange("(ks p) f -> p ks f", p=P)
        if _KSTOP >= 9:
         with (
            nc.named_scope("ff1"),
            tc.tile_pool(name="wff1", bufs=2) as wff1_pool,
            tc.tile_pool(name="ff1_ps", bufs=2, space="PSUM") as ps_ff1,
            tc.tile_pool(name="hT", bufs=3) as hT_pool,
            tc.tile_pool(name="gT", bufs=3) as gT_pool,
         ):
            def _load3(c0, sfx):
                w16 = wff1_pool.tile([P, DSUB, D], fp8, tag="w16" + sfx)
                nc.sync.dma_start(w16, wff1_t[:, :, c0:c0 + D])
                wlo = wff1_pool.tile([P, DSUB, D], fp8, tag="wlo" + sfx)
                nc.sync.dma_start(wlo, wff1l_t[:, :, c0:c0 + D])
                whi = wff1_pool.tile([P, DSUB, D], fp8, tag="whi" + sfx)
                nc.sync.dma_start(whi, wff1h_t[:, :, c0:c0 + D])
                return w16, wlo, whi

            def _ff1_mm(ps, w3, fi, rhs_hi, rhs_lo):
                w16, wlo, whi = w3
                _proj_dr(nc, ps, w16, rhs_hi, fi, SQ, last=False)
                _proj_dr(nc, ps, wlo, rhs_hi, fi, SQ, first=False, last=False)
                _proj_dr(nc, ps, whi, rhs_lo, fi, SQ, first=False)

            for cc in range(4):
                wh3 = _load3(cc * D, "h")
                wg3 = _load3(FF + cc * D, "g")
                for fi in range(DSUB):
                    fc = cc * DSUB + fi
                    ps_h = ps_ff1.tile([P, SQ], f32, tag="psh")
                    _ff1_mm(ps_h, wh3, fi, xn3T8, xn3loT8)
                    ps_g = ps_ff1.tile([P, SQ], f32, tag="psg")
                    _ff1_mm(ps_g, wg3, fi, xn3T8, xn3loT8)
                    hT = hT_pool.tile([P, SQ], bf16, tag="hT")
                    nc.vector.tensor_scalar(
                        hT, ps_h, cinv16, bffh_s[:, fc:fc + 1],
                        ALU.mult, ALU.add)
                    gT = gT_pool.tile([P, SQ], bf16, tag="gT")
                    nc.scalar.activation(
                        gT, ps_g, AF.Gelu, bias=bffg_s[:, fc:fc + 1],
                        scale=1.0 / (SW * 16.0))
                    eng = nc.gpsimd if fc % 2 else nc.vector
                    eng.tensor_tensor(mT8[:, fc, :], hT, gT, ALU.mult)

        if _KSTOP >= 10:
         with (
            nc.named_scope("ff2"),
            tc.tile_pool(name="ff2_ps", bufs=1, space="PSUM") as ps_ff2,
         ):
            ps_o = [ps_ff2.tile([P, 512], f32, tag=f"o{i}", name=f"ps_o{i}")
                    for i in range(8)]
            for ksp in range(FSUB // 2):
                for sc in range(QSUB):
                    for dh in range(2):
                        nc.tensor.matmul(
                            ps_o[sc * 2 + dh],
                            mT8[:, 2 * ksp:2 * ksp + 2, sc * P:(sc + 1) * P],
                            wff2_sb[:, 2 * ksp:2 * ksp + 2,
                                    dh * 512:(dh + 1) * 512],
                            start=(ksp == 0), stop=(ksp == FSUB // 2 - 1),
                            perf_mode=DR)
            for sc in range(QSUB):
                if with_bff2:
                    nc.gpsimd.tensor_tensor(
                        hbuf[:, sc], hbuf[:, sc], bff2_b, ALU.add)
                for dh in range(2):
                    sl = slice(dh * 512, (dh + 1) * 512)
                    nc.vector.tensor_tensor(
                        hbuf[:, sc, sl], ps_o[sc * 2 + dh],
                        hbuf[:, sc, sl], ALU.add)
        for sc in range(QSUB):
            nc.sync.dma_start(
                out_dr.rearrange("(ss p) d -> p ss d", p=P)[:, sc],
                hbuf[:, sc])
        free_wff2(); free_mT8()
        free_xn3loT8(); free_xn3T8(); free_hbuf()

    nc.compile()
    return nc


# --------------------------------------------------------------------------
# host side
# --------------------------------------------------------------------------

_NC = None
_NC_FLAGS = None


def _perm_qk():
    """Column permutation for Wq/Wk: head-interleaved DoubleRow layout."""
    perm = np.empty(D, np.int64)
    for h in range(H):
        for dl in range(DH):
            j = (h // 4) * 2 + (dl // 32)
            c = (h % 4) * 32 + (dl % 32)
            perm[j * P + c] = h * DH + dl
    return perm


_PERM = _perm_qk()


def _q8(x, scale=1.0):
    x = np.asarray(x, np.float32) * scale
    am = np.abs(x).max()
    assert am < 440, f"fp8 overflow: absmax {am}"
    return x.astype(FP8T)


def _get_nc(flags=()):
    global _NC, _NC_FLAGS
    flags = tuple(flags)
    if _NC is None or _NC_FLAGS != flags:
        _NC = build_nc(**dict(flags))
        _NC_FLAGS = flags
    return _NC


def _make_in_maps(inputs):
    f = np.float32
    hidden = np.asarray(inputs["hidden_states"], f)
    context = np.asarray(inputs["context"], f)
    mask = np.asarray(inputs["encoder_key_padding_mask"]).astype(f)
    g1, b1 = np.asarray(inputs["g1"], f), np.asarray(inputs["b1"], f)
    g2, b2 = np.asarray(inputs["g2"], f), np.asarray(inputs["b2"], f)
    g3, b3 = np.asarray(inputs["g3"], f), np.asarray(inputs["b3"], f)

    def fold(g, W):
        return g[:, None] * np.asarray(W, f)

    Wq1 = fold(g1, inputs["Wq1"])[:, _PERM]
    Wk1 = fold(g1, inputs["Wk1"])[:, _PERM]
    Wv1 = fold(g1, inputs["Wv1"])
    Wo1 = np.asarray(inputs["Wo1"], f)
    qb1 = (b1 @ np.asarray(inputs["Wq1"], f))[_PERM]
    kb1 = (b1 @ np.asarray(inputs["Wk1"], f))[_PERM]
    vb1 = b1 @ np.asarray(inputs["Wv1"], f)
    Wq2 = fold(g2, inputs["Wq2"])[:, _PERM]
    Wk2 = np.asarray(inputs["Wk2"], f)[:, _PERM]
    Wv2 = np.asarray(inputs["Wv2"], f)
    Wo2 = np.asarray(inputs["Wo2"], f)
    qb2 = (b2 @ np.asarray(inputs["Wq2"], f))[_PERM]
    Wff1 = fold(g3, inputs["Wff1"])
    bff1 = np.asarray(inputs["bff1"], f) + b3 @ np.asarray(inputs["Wff1"], f)
    Wff2 = np.asarray(inputs["Wff2"], f)
    bo1 = np.asarray(inputs["bo1"], f)
    bo2 = np.asarray(inputs["bo2"], f)
    bff2 = np.asarray(inputs["bff2"], f)

    flags = (
        ("with_vb1", bool(np.any(vb1))),
        ("with_bo1", bool(np.any(bo1))),
        ("with_bo2", bool(np.any(bo2))),
        ("with_bff2", bool(np.any(bff2))),
        ("with_qb1", bool(np.any(qb1))),
        ("with_kb1", bool(np.any(kb1))),
        ("with_qb2", bool(np.any(qb2))),
    )

    wff1_hi = _q8(Wff1, SW)
    wff1_hi_f = wff1_hi.astype(f)
    wff1_hi16 = _q8(wff1_hi_f, 16.0)          # exact exponent shift
    wff1_lo = _q8((SW * Wff1 - wff1_hi_f), 16.0)

    shared = {
        "Wq1": _q8(Wq1, SW), "Wk1": _q8(Wk1, SW), "Wv1": _q8(Wv1, SW),
        "Wo1": _q8(Wo1, SW),
        "Wq2": _q8(Wq2, SW), "Wk2": _q8(Wk2, SW), "Wv2": _q8(Wv2, SW),
        "Wo2": _q8(Wo2, SW),
        "Wff1": wff1_hi16, "Wff1l": wff1_lo, "Wff1h": wff1_hi,
        "Wff2": _q8(Wff2, 64.0),
        "qb1_16": np.ascontiguousarray(SW * qb1),
        "kb1_16": np.ascontiguousarray(SW * kb1),
        "qb2_16": np.ascontiguousarray(SW * qb2),
        "vb1_16": np.ascontiguousarray(SW * vb1),
        "bo1_s": np.ascontiguousarray(SH * bo1),
        "bo2_s": np.ascontiguousarray(SH * bo2),
        "bff2_s": np.ascontiguousarray(SH * bff2),
        "bffh_16": np.ascontiguousarray(SW * bff1[:FF]),
        "bffg": np.ascontiguousarray(bff1[FF:]),
    }

    in_maps = []
    for core in range(NCORES):
        b, q = core // 2, core % 2
        x = hidden[b] if q == 0 else np.roll(hidden[b], -SQ, axis=0)
        ctxT = np.ascontiguousarray(context[b].T)
        ctxTm = np.ascontiguousarray((mask[b][:, None] * context[b]).T)
        in_maps.append({
            **shared,
            "x": np.ascontiguousarray(x),
            "ctxT8": _q8(ctxT),
            "ctxTm8": _q8(ctxTm),
            "mcol": np.ascontiguousarray(C64 * mask[b]),
        })
    return in_maps, flags


def run(inputs, **spmd_kwargs):
    in_maps, flags = _make_in_maps(inputs)
    res = run_bass_kernel_spmd(
        _get_nc(flags), in_maps, core_ids=list(range(NCORES)),
        **spmd_kwargs)
    out = np.empty((B, S, D), np.float32)
    for core in range(NCORES):
        b, q = core // 2, core % 2
        out[b, q * SQ:(q + 1) * SQ] = res.results[core]["out"] * (1.0 / SH)
    return out, res


def kernel(**inputs):
    out, _ = run(inputs)
    return out
